# revision 16
# baseline (speedup 1.0000x reference)
"""Trainium2 Bass kernel for nn_DirectedMessage (gnn_message_passing).

Math: the reference's per-angle tensor m_and_e depends only on kj_idx[a], so
    final[e] = h(e) * S(e)
      h(e) = (silu(m_ji[e] @ W_m.T + b) * (e_rbf[e] @ W_e.T)) @ final_w.T   [E, 6]
      s[a] = a_sbf[a] . sum_r W_a[r]                                        [A]
      S(e) = segment_sum(s, kj_idx)[e]                                      [E]

Distribution (owner-computes): edges are sharded contiguously across the 8
cores; each angle is routed (on host, as part of sharding) to the core that
owns its kj edge, so no collective is needed.  Within a core, angles are
binned into fixed 64-edge windows (tile t covers local edges [64t, 64t+64));
the device computes s on-chip and performs the segment-sum with one small
PSUM-accumulating matmul per tile (lhsT = constant ones column, rhs = a
one-hot-times-s matrix built on the vector engine).  Overflow angles (>128
in one window) go through 4 generic full-width scatter tiles.
"""

import sys
import types

sys.path.insert(0, "/opt/trn_rl_repo")

# Optional NTFF trace hook (lets BASS_TRACE=1 capture hardware profiles).
try:  # pragma: no cover
    import trn_agent_boot.trn_boot as _tb

    if "antenv.axon_hooks" not in sys.modules:
        _hook = _tb._ntff_profile_via_ctypes("/opt/axon/libaxon_pjrt.so")
        _m = types.ModuleType("antenv.axon_hooks")
        _m.get_axon_ntff_profile_hook = lambda: _hook
        sys.modules["antenv.axon_hooks"] = _m
except Exception:
    pass

import os

import numpy as np

import concourse.bacc as bacc
import concourse.mybir as mybir
import concourse.tile as tile
from concourse.bass_utils import run_bass_kernel_spmd

F16 = mybir.dt.float16
F32 = mybir.dt.float32
OP = mybir.AluOpType
ACTF = mybir.ActivationFunctionType
AX = mybir.AxisListType

E = 400000
A = 600000
CAT = 134
NRBF = 6
ADIM = 42
NCORES = 8
ESH = E // NCORES          # 50000 edges per core
PW = 392                   # legacy name; see NT/PC below
EP = 50176                 # padded edges per core
PC = 784                   # S columns; e_local = c*64 + p, p in [0,64)
NT = EP // 64              # 784 primary scatter tiles (64-edge windows)
NG = 4                     # generic (overflow) scatter tiles
NSLOT = NT + NG            # 788 angle slot columns
SUP = 2048                 # edge super-block (columns per DMA)
BLK = 512                  # matmul moving width
PH1_CH = 16                # angle slot columns per phase-1 chunk

_PROG = None
LAST_RESULT = None


def _build_program():
    # CoreSim has no Silu; tests can force Sigmoid to validate dataflow.
    silu_f = (ACTF.Sigmoid if os.environ.get("KERNEL_SIM_ACT") == "sigmoid"
              else ACTF.Silu)
    nc = bacc.Bacc("TRN2", target_bir_lowering=False, debug=False,
                   num_devices=NCORES)

    a_d = nc.dram_tensor("a_arr", [128, NSLOT * ADIM], F16, kind="ExternalInput")
    r_d = nc.dram_tensor("r16", [128, NSLOT], F16, kind="ExternalInput")
    cg_d = nc.dram_tensor("cg16", [128, NG], F32, kind="ExternalInput")
    mT_d = nc.dram_tensor("mT0", [128, EP], F16, kind="ExternalInput")
    tail_d = nc.dram_tensor("tailT", [12, EP], F16, kind="ExternalInput")
    wm00_d = nc.dram_tensor("Wm00", [128, 128], F16, kind="ExternalInput")
    wmk1_d = nc.dram_tensor("WmK1M0", [6, 128], F16, kind="ExternalInput")
    wsa_d = nc.dram_tensor("WstragA", [128, 38], F16, kind="ExternalInput")
    wsb_d = nc.dram_tensor("WstragB", [12, 38], F16, kind="ExternalInput")
    we0_d = nc.dram_tensor("WeT0", [6, 128], F16, kind="ExternalInput")
    fw0_d = nc.dram_tensor("fw0", [128, 6], F16, kind="ExternalInput")
    fw1_d = nc.dram_tensor("fw1", [6, 6], F16, kind="ExternalInput")
    b0_d = nc.dram_tensor("b0", [128, 1], F32, kind="ExternalInput")
    b1_d = nc.dram_tensor("b1", [6, 1], F32, kind="ExternalInput")
    wsum_d = nc.dram_tensor("wsum_rep", [128, PH1_CH * ADIM], F16,
                            kind="ExternalInput")
    io64_d = nc.dram_tensor("iota8x64", [128, 512], F16, kind="ExternalInput")
    io128_d = nc.dram_tensor("iota128", [128, 128], F16, kind="ExternalInput")
    io392_d = nc.dram_tensor("iota784", [128, PC], F16, kind="ExternalInput")
    ones_d = nc.dram_tensor("ones128", [128, 1], F16, kind="ExternalInput")
    ident_d = nc.dram_tensor("ident128", [128, 128], F32, kind="ExternalInput")
    out_d = nc.dram_tensor("out", [6, EP], F32, kind="ExternalOutput")

    with tile.TileContext(nc) as tc:
        with tc.tile_pool(name="const", bufs=1) as cpool, \
             tc.tile_pool(name="dram", bufs=1, space="DRAM") as dpool, \
             tc.tile_pool(name="persist", bufs=1) as ppool:

            def cload(dram, shape, dtype=F16, tag=None):
                t = cpool.tile(shape, dtype, tag=tag or dram.name)
                nc.sync.dma_start(out=t[:], in_=dram[:])
                return t

            r_t = cload(r_d, [128, NSLOT])
            cg_t = cload(cg_d, [128, NG], F32)
            wm00_t = cload(wm00_d, [128, 128])
            wmk1_t = cload(wmk1_d, [6, 128])
            wsa_t = cload(wsa_d, [128, 38])
            wsb_t = cload(wsb_d, [12, 38])
            we0_t = cload(we0_d, [6, 128])
            fw0_t = cload(fw0_d, [128, 6])
            fw1_t = cload(fw1_d, [6, 6])
            b0_t = cload(b0_d, [128, 1], F32)
            b1_t = cload(b1_d, [6, 1], F32)
            wsum_t = cload(wsum_d, [128, PH1_CH * ADIM])
            io64_t = cload(io64_d, [128, 512])
            io128_t = cload(io128_d, [128, 128])
            io392_t = cload(io392_d, [128, PC])
            ones_t = cload(ones_d, [128, 1])
            ident_t = cload(ident_d, [128, 128], F32)

            s_all = ppool.tile([128, NSLOT], F32, tag="s_all")
            s16 = ppool.tile([128, NSLOT], F16, tag="s16")
            S_sb = ppool.tile([64, PC], F32, tag="S_sb")
            S_sbT = ppool.tile([98, 512], F32, tag="S_sbT")
            S_dramT = dpool.tile([PC, 64], F32, tag="S_dramT")

            # ---- Phase 1: s[slot] = a_sbf[slot] . w_sum  -------------------
            with tc.tile_pool(name="ph1", bufs=3) as p1:
                for off in range(0, NSLOT, PH1_CH):
                    w = min(PH1_CH, NSLOT - off)
                    at = p1.tile([128, PH1_CH * ADIM], F16, tag="at")
                    nc.sync.dma_start(out=at[:, :w * ADIM],
                                      in_=a_d[:, off * ADIM:(off + w) * ADIM])
                    pr = p1.tile([128, PH1_CH * ADIM], F16, tag="pr")
                    nc.gpsimd.tensor_tensor(out=pr[:, :w * ADIM],
                                            in0=at[:, :w * ADIM],
                                            in1=wsum_t[:, :w * ADIM],
                                            op=OP.mult)
                    nc.vector.tensor_reduce(
                        out=s_all[:, off:off + w],
                        in_=pr[:, :w * ADIM].rearrange("p (t d) -> p t d",
                                                       d=ADIM),
                        axis=AX.X, op=OP.add)
            nc.vector.tensor_copy(out=s16[:], in_=s_all[:])

            # ---- Phase 2: S[p, c] = segment-sum of s ----------------------
            with tc.tile_pool(name="ph2psum", bufs=1, space="PSUM") as sp, \
                 tc.tile_pool(name="ph2", bufs=4) as p2:
                S_ps = sp.tile([64, PC], F32, tag="S_ps")
                # One accumulation group per PSUM bank: a zeroing matmul
                # (start=True) covers the bank, every scatter matmul below
                # accumulates into it (start=False), the last generic
                # matmul per bank carries stop=True.
                z1 = p2.tile([1, 64], F16, tag="z1")
                nc.gpsimd.memset(z1[:], 0)
                nc.tensor.matmul(S_ps[0:64, 0:512], z1[:], io64_t[0:1, :],
                                 start=True, stop=False)
                nc.tensor.matmul(S_ps[0:64, 512:PC], z1[:],
                                 io64_t[0:1, 0:PC - 512],
                                 start=True, stop=False)
                B = 8
                # Primary tile t covers local edges [64t, 64t+64); with
                # el = c*64 + p this is psum column t, partitions 0:64.
                for grp in range(NT // B):
                    eq8 = p2.tile([128, B * 64], F16, tag="eq8")
                    nc.vector.tensor_tensor(
                        out=eq8[:].rearrange("p (t j) -> p t j", j=64),
                        in0=io64_t[:].rearrange("p (t j) -> p t j", j=64),
                        in1=r_t[:, grp * B:(grp + 1) * B]
                            .to_broadcast([128, B, 64]),
                        op=OP.is_equal)
                    pone8 = p2.tile([128, B * 64], F16, tag="pone8")
                    nc.vector.tensor_tensor(
                        out=pone8[:].rearrange("p (t j) -> p t j", j=64),
                        in0=eq8[:].rearrange("p (t j) -> p t j", j=64),
                        in1=s16[:, grp * B:(grp + 1) * B]
                            .to_broadcast([128, B, 64]),
                        op=OP.mult)
                    for i in range(B):
                        t = grp * B + i
                        nc.tensor.matmul(
                            S_ps[0:64, t:t + 1],
                            pone8[:, i * 64:(i + 1) * 64], ones_t[:],
                            start=False, stop=False)
                for g in range(NG):
                    pg = p2.tile([128, 64], F16, tag="pg")
                    nc.vector.scalar_tensor_tensor(
                        out=pg[:], in0=io64_t[:, 0:64],
                        scalar=r_t[:, NT + g:NT + g + 1],
                        in1=s16[:, NT + g:NT + g + 1].to_broadcast([128, 64]),
                        op0=OP.is_equal, op1=OP.mult)
                    cg = p2.tile([128, PC], F16, tag="cgt")
                    nc.vector.tensor_scalar(
                        out=cg[:], in0=io392_t[:],
                        scalar1=cg_t[:, g:g + 1], scalar2=None,
                        op0=OP.is_equal)
                    nc.tensor.matmul(S_ps[0:64, 0:512], pg[:], cg[:, 0:512],
                                     start=False, stop=(g == NG - 1))
                    nc.tensor.matmul(S_ps[0:64, 512:PC], pg[:],
                                     cg[:, 512:PC],
                                     start=False, stop=(g == NG - 1))
                nc.scalar.activation(out=S_sb[:], in_=S_ps[:], func=ACTF.Copy)
                # Transpose S [p, c] -> [c, p] so DRAM flat order == el order.
                for q in range(8):
                    T_ps = sp.tile([98, 64], F32, tag="T_ps")
                    nc.tensor.transpose(out=T_ps[:],
                                        in_=S_sb[0:64, 98 * q:98 * (q + 1)],
                                        identity=ident_t[0:64, 0:64])
                    nc.scalar.activation(out=S_sbT[:, 64 * q:64 * (q + 1)],
                                         in_=T_ps[:], func=ACTF.Copy)
            nc.sync.dma_start(
                out=S_dramT[:].rearrange("(q r) p -> r q p", q=8),
                in_=S_sbT[:].rearrange("r (q p) -> r q p", q=8))
            S_flat = S_dramT[:].rearrange("(o c) p -> o (c p)", o=1)

            # ---- Phase 3: h(e) and output ---------------------------------
            with tc.tile_pool(name="pm", bufs=2, space="PSUM") as pmp, \
                 tc.tile_pool(name="st", bufs=2, space="PSUM") as stp, \
                 tc.tile_pool(name="te", bufs=2, space="PSUM") as tep, \
                 tc.tile_pool(name="pj", bufs=2, space="PSUM") as pjp, \
                 tc.tile_pool(name="ph3", bufs=2) as p3:
                supers = [(st0, min(SUP, EP - st0))
                          for st0 in range(0, EP, SUP)]
                for st0, wd in supers:
                    mt = p3.tile([128, SUP], F16, tag="mt")
                    nc.sync.dma_start(out=mt[:, :wd], in_=mT_d[:, st0:st0 + wd])
                    tl = p3.tile([12, SUP], F16, tag="tl")
                    nc.sync.dma_start(out=tl[:, :wd],
                                      in_=tail_d[:, st0:st0 + wd])
                    et = p3.tile([6, SUP], F16, tag="et")
                    nc.sync.dma_start(out=et[:, :wd],
                                      in_=tail_d[6:12, st0:st0 + wd])
                    s6 = p3.tile([6, SUP], F32, tag="s6")
                    nc.sync.dma_start(out=s6[:, :wd],
                                      in_=S_flat[0:1, st0:st0 + wd]
                                          .to_broadcast([6, wd]))
                    ob = p3.tile([6, SUP], F32, tag="ob")
                    for lo in range(0, wd, BLK):
                        sl = slice(lo, lo + BLK)
                        pm0 = pmp.tile([128, BLK], F32, tag="pm0")
                        nc.tensor.matmul(pm0[:], wm00_t[:], mt[:, sl],
                                         start=True, stop=False)
                        nc.tensor.matmul(pm0[:], wmk1_t[:], tl[0:6, sl],
                                         start=False, stop=True)
                        stg = stp.tile([38, BLK], F32, tag="stg")
                        nc.tensor.matmul(stg[:], wsa_t[:], mt[:, sl],
                                         start=True, stop=False)
                        nc.tensor.matmul(stg[:], wsb_t[:], tl[0:12, sl],
                                         start=False, stop=True)
                        te0 = tep.tile([128, BLK], F32, tag="te0")
                        nc.tensor.matmul(te0[:], we0_t[:], et[:, sl],
                                         start=True, stop=True)
                        m0 = p3.tile([128, BLK], F16, tag="m0")
                        nc.scalar.activation(out=m0[:], in_=pm0[:],
                                             func=silu_f,
                                             bias=b0_t[:, 0:1])
                        m1 = p3.tile([6, BLK], F16, tag="m1")
                        nc.scalar.activation(out=m1[:], in_=stg[0:6, :],
                                             func=silu_f,
                                             bias=b1_t[:, 0:1])
                        t0 = p3.tile([128, BLK], F16, tag="t0")
                        nc.scalar.activation(out=t0[:], in_=te0[:],
                                             func=ACTF.Copy)
                        me0 = p3.tile([128, BLK], F16, tag="me0")
                        nc.gpsimd.tensor_tensor(out=me0[:], in0=m0[:],
                                                in1=t0[:], op=OP.mult)
                        me1 = p3.tile([6, BLK], F16, tag="me1")
                        nc.vector.tensor_tensor(out=me1[:], in0=m1[:],
                                                in1=stg[32:38, :], op=OP.mult)
                        pj = pjp.tile([6, BLK], F32, tag="pj")
                        nc.tensor.matmul(pj[:], fw0_t[:], me0[:],
                                         start=True, stop=False)
                        nc.tensor.matmul(pj[:], fw1_t[:], me1[:],
                                         start=False, stop=True)
                        nc.vector.tensor_tensor(out=ob[:, sl], in0=pj[:],
                                                in1=s6[:, sl], op=OP.mult)
                    nc.sync.dma_start(out=out_d[:, st0:st0 + wd],
                                      in_=ob[:, :wd])

    nc.compile()
    return nc


def _get_program():
    global _PROG
    if _PROG is None:
        _PROG = _build_program()
    return _PROG


def _f16(x):
    return np.ascontiguousarray(x, dtype=np.float16)


def _prep_inputs(m_ji, e_rbf, a_sbf, kj_idx, W_m, b_m, W_e, W_a, final_w):
    m_ji = np.asarray(m_ji, dtype=np.float32)
    e_rbf = np.asarray(e_rbf, dtype=np.float32)
    a_sbf = np.asarray(a_sbf, dtype=np.float32)
    kj = np.asarray(kj_idx).astype(np.int64).ravel()
    W_m = np.asarray(W_m, dtype=np.float32)
    b_m = np.asarray(b_m, dtype=np.float32).ravel()
    W_e = np.asarray(W_e, dtype=np.float32)
    W_a = np.asarray(W_a, dtype=np.float32)
    fw = np.asarray(final_w, dtype=np.float32)

    # shared weight-derived inputs
    WmT = W_m.T  # [c_in, c_out]
    WeT = W_e.T  # [6?? no: W_e is [CAT, NRBF]] -> W_e.T is [NRBF, CAT]
    fwT = fw.T   # [CAT, NRBF]
    wsa = np.zeros((128, 38), np.float32)
    wsa[:, 0:6] = WmT[:128, 128:134]
    wsb = np.zeros((12, 38), np.float32)
    wsb[0:6, 0:6] = WmT[128:134, 128:134]
    # te[c_out, e] = sum_j W_e[c_out, j] * e_rbf[e, j]; lhsT[k=j, m=c_out]
    # = W_e.T[j, c_out] = WeT[j, c_out] with WeT = W_e.T  ([NRBF, CAT])
    # te tail lands on psum partitions 32:38 (32-aligned engine reads).
    wsb[6:12, 32:38] = WeT[:, 128:134]
    w_sum = W_a.sum(axis=0)  # [42]

    shared = {
        "Wm00": _f16(WmT[:128, :128]),
        "WmK1M0": _f16(WmT[128:134, :128]),
        "WstragA": _f16(wsa),
        "WstragB": _f16(wsb),
        "WeT0": _f16(WeT[:, :128]),
        "fw0": _f16(fwT[:128, :]),
        "fw1": _f16(fwT[128:134, :]),
        "b0": np.ascontiguousarray(b_m[:128, None], np.float32),
        "b1": np.ascontiguousarray(b_m[128:134, None], np.float32),
        "wsum_rep": _f16(np.tile(w_sum, (128, PH1_CH))),
        "iota8x64": _f16(np.tile(np.arange(64), (128, 8))),
        "iota128": _f16(np.tile(np.arange(128), (128, 1))),
        "iota784": _f16(np.tile(np.arange(PC), (128, 1))),
        "ones128": _f16(np.ones((128, 1))),
        "ident128": np.eye(128, dtype=np.float32),
    }

    order = np.argsort(kj, kind="stable")
    kj_s = kj[order]
    a_s = a_sbf[order]
    bounds = np.searchsorted(kj_s, np.arange(NCORES + 1) * ESH)

    in_maps = []
    for i in range(NCORES):
        lo, hi = int(bounds[i]), int(bounds[i + 1])
        el = kj_s[lo:hi] - i * ESH          # sorted ascending in [0, ESH)
        av = a_s[lo:hi]
        n = el.shape[0]
        t_of = el // 64
        starts = np.searchsorted(t_of, np.arange(NT))
        rank = np.arange(n) - starts[t_of]
        prim = rank < 128
        r16 = np.zeros((128, NSLOT), np.float16)
        a_arr = np.zeros((128, NSLOT, ADIM), np.float16)
        cg16 = np.zeros((128, NG), np.float32)
        r16[rank[prim], t_of[prim]] = (el % 64)[prim]
        a_arr[rank[prim], t_of[prim], :] = av[prim]
        n_sp = int((~prim).sum())
        if n_sp > NG * 128:
            raise RuntimeError(f"core {i}: {n_sp} spill angles > {NG*128}")
        if n_sp:
            sp_el = el[~prim]
            sp_idx = np.arange(n_sp)
            rr, cc = sp_idx % 128, sp_idx // 128
            r16[rr, NT + cc] = sp_el % 64
            cg16[rr, cc] = sp_el // 64
            a_arr[rr, NT + cc, :] = av[~prim]

        msl = m_ji[i * ESH:(i + 1) * ESH]
        mT = np.zeros((CAT, EP), np.float16)
        mT[:, :ESH] = msl.T
        esl = e_rbf[i * ESH:(i + 1) * ESH]
        tailT = np.zeros((12, EP), np.float16)
        tailT[0:6, :ESH] = msl.T[128:134]
        tailT[6:12, :ESH] = esl.T

        im = dict(shared)
        im["a_arr"] = np.ascontiguousarray(a_arr.reshape(128, NSLOT * ADIM))
        im["r16"] = np.ascontiguousarray(r16)
        im["cg16"] = np.ascontiguousarray(cg16)
        im["mT0"] = np.ascontiguousarray(mT[:128])
        im["tailT"] = np.ascontiguousarray(tailT)
        in_maps.append(im)
    return in_maps


def kernel(m_ji, nbr_list, angle_list, e_rbf, a_sbf, kj_idx,
           W_m, b_m, W_e, W_a, final_w):
    global LAST_RESULT
    in_maps = _prep_inputs(m_ji, e_rbf, a_sbf, kj_idx, W_m, b_m, W_e, W_a,
                           final_w)
    nc = _get_program()
    res = run_bass_kernel_spmd(nc, in_maps, core_ids=list(range(NCORES)))
    LAST_RESULT = res
    out = np.empty((E, NRBF), np.float32)
    for i in range(NCORES):
        out[i * ESH:(i + 1) * ESH] = res.results[i]["out"][:, :ESH].T
    return out


# revision 20
# speedup vs baseline: 1.0793x; 1.0793x over previous
"""Trainium2 Bass kernel for nn_DirectedMessage (gnn_message_passing).

Math: the reference's per-angle tensor m_and_e depends only on kj_idx[a], so
    final[e] = h(e) * S(e)
      h(e) = (silu(m_ji[e] @ W_m.T + b) * (e_rbf[e] @ W_e.T)) @ final_w.T   [E, 6]
      s[a] = a_sbf[a] . sum_r W_a[r]                                        [A]
      S(e) = segment_sum(s, kj_idx)[e]                                      [E]

Distribution (owner-computes): edges are sharded contiguously across the 8
cores; each angle is routed (on host, as part of sharding) to the core that
owns its kj edge, so no collective is needed.  Within a core, angles are
binned into fixed 64-edge windows (tile t covers local edges [64t, 64t+64));
the device computes s on-chip and performs the segment-sum with one small
PSUM-accumulating matmul per tile (lhsT = constant ones column, rhs = a
one-hot-times-s matrix built on the vector engine).  Overflow angles (>128
in one window) go through 4 generic full-width scatter tiles.
"""

import sys
import types

sys.path.insert(0, "/opt/trn_rl_repo")

# Optional NTFF trace hook (lets BASS_TRACE=1 capture hardware profiles).
try:  # pragma: no cover
    import trn_agent_boot.trn_boot as _tb

    if "antenv.axon_hooks" not in sys.modules:
        _hook = _tb._ntff_profile_via_ctypes("/opt/axon/libaxon_pjrt.so")
        _m = types.ModuleType("antenv.axon_hooks")
        _m.get_axon_ntff_profile_hook = lambda: _hook
        sys.modules["antenv.axon_hooks"] = _m
except Exception:
    pass

import os

import numpy as np

import concourse.bacc as bacc
import concourse.mybir as mybir
import concourse.tile as tile
from concourse.bass_utils import run_bass_kernel_spmd

F16 = mybir.dt.float16
F32 = mybir.dt.float32
OP = mybir.AluOpType
ACTF = mybir.ActivationFunctionType
AX = mybir.AxisListType

E = 400000
A = 600000
CAT = 134
NRBF = 6
ADIM = 42
NCORES = 8
ESH = E // NCORES          # 50000 edges per core
PW = 392                   # legacy name; see NT/PC below
EP = 50176                 # padded edges per core
PC = 784                   # S columns; e_local = c*64 + p, p in [0,64)
NT = EP // 64              # 784 primary scatter tiles (64-edge windows)
NG = 4                     # generic (overflow) scatter tiles
NSLOT = NT + NG            # 788 angle slot columns
SUP = 2048                 # edge super-block (columns per DMA)
BLK = 512                  # matmul moving width
PH1_CH = 16                # angle slot columns per phase-1 chunk

_PROG = None
LAST_RESULT = None


def _build_program():
    # CoreSim has no Silu; tests can force Sigmoid to validate dataflow.
    silu_f = (ACTF.Sigmoid if os.environ.get("KERNEL_SIM_ACT") == "sigmoid"
              else ACTF.Silu)
    nc = bacc.Bacc("TRN2", target_bir_lowering=False, debug=False,
                   num_devices=NCORES)

    a_d = nc.dram_tensor("a_arr", [128, NSLOT * ADIM], F16, kind="ExternalInput")
    r_d = nc.dram_tensor("r16", [128, NSLOT], F32, kind="ExternalInput")
    cg_d = nc.dram_tensor("cg16", [128, NG], F32, kind="ExternalInput")
    mT_d = nc.dram_tensor("mT0", [128, EP], F16, kind="ExternalInput")
    tail_d = nc.dram_tensor("tailT", [12, EP], F16, kind="ExternalInput")
    wm00_d = nc.dram_tensor("Wm00", [128, 128], F16, kind="ExternalInput")
    wmk1_d = nc.dram_tensor("WmK1M0", [6, 128], F16, kind="ExternalInput")
    wsa_d = nc.dram_tensor("WstragA", [128, 38], F16, kind="ExternalInput")
    wsb_d = nc.dram_tensor("WstragB", [12, 38], F16, kind="ExternalInput")
    we0_d = nc.dram_tensor("WeT0", [6, 128], F16, kind="ExternalInput")
    fw0_d = nc.dram_tensor("fw0", [128, 6], F16, kind="ExternalInput")
    fw1_d = nc.dram_tensor("fw1", [6, 6], F16, kind="ExternalInput")
    b0_d = nc.dram_tensor("b0", [128, 1], F32, kind="ExternalInput")
    b1_d = nc.dram_tensor("b1", [6, 1], F32, kind="ExternalInput")
    wsum_d = nc.dram_tensor("wsum_rep", [128, PH1_CH * ADIM], F16,
                            kind="ExternalInput")
    io64_d = nc.dram_tensor("iota8x64", [128, 512], F16, kind="ExternalInput")
    io128_d = nc.dram_tensor("iota128", [128, 128], F16, kind="ExternalInput")
    io392_d = nc.dram_tensor("iota784", [128, PC], F16, kind="ExternalInput")
    ones_d = nc.dram_tensor("ones128", [128, 1], F16, kind="ExternalInput")
    ident_d = nc.dram_tensor("ident128", [128, 128], F32, kind="ExternalInput")
    out_d = nc.dram_tensor("out", [6, EP], F16, kind="ExternalOutput")

    with tile.TileContext(nc) as tc:
        with tc.tile_pool(name="const", bufs=1) as cpool, \
             tc.tile_pool(name="dram", bufs=1, space="DRAM") as dpool, \
             tc.tile_pool(name="persist", bufs=1) as ppool:

            def cload(dram, shape, dtype=F16, tag=None):
                t = cpool.tile(shape, dtype, tag=tag or dram.name)
                nc.sync.dma_start(out=t[:], in_=dram[:])
                return t

            r_t = cload(r_d, [128, NSLOT], F32)
            cg_t = cload(cg_d, [128, NG], F32)
            wm00_t = cload(wm00_d, [128, 128])
            wmk1_t = cload(wmk1_d, [6, 128])
            wsa_t = cload(wsa_d, [128, 38])
            wsb_t = cload(wsb_d, [12, 38])
            we0_t = cload(we0_d, [6, 128])
            fw0_t = cload(fw0_d, [128, 6])
            fw1_t = cload(fw1_d, [6, 6])
            b0_t = cload(b0_d, [128, 1], F32)
            b1_t = cload(b1_d, [6, 1], F32)
            wsum_t = cload(wsum_d, [128, PH1_CH * ADIM])
            io64_t = cload(io64_d, [128, 512])
            io128_t = cload(io128_d, [128, 128])
            io392_t = cload(io392_d, [128, PC])
            ones_t = cload(ones_d, [128, 1])
            ident_t = cload(ident_d, [128, 128], F32)

            s_all = ppool.tile([128, NSLOT], F32, tag="s_all")
            s16 = ppool.tile([128, NSLOT], F16, tag="s16")
            S_sb = ppool.tile([64, PC], F32, tag="S_sb")
            S_sbT = ppool.tile([98, 512], F16, tag="S_sbT")
            S_dramT = dpool.tile([PC, 64], F16, tag="S_dramT")

            # ---- Phase 1: s[slot] = a_sbf[slot] . w_sum  -------------------
            with tc.tile_pool(name="ph1", bufs=3) as p1:
                for off in range(0, NSLOT, PH1_CH):
                    w = min(PH1_CH, NSLOT - off)
                    at = p1.tile([128, PH1_CH * ADIM], F16, tag="at")
                    nc.sync.dma_start(out=at[:, :w * ADIM],
                                      in_=a_d[:, off * ADIM:(off + w) * ADIM])
                    pr = p1.tile([128, PH1_CH * ADIM], F16, tag="pr")
                    nc.gpsimd.tensor_tensor(out=pr[:, :w * ADIM],
                                            in0=at[:, :w * ADIM],
                                            in1=wsum_t[:, :w * ADIM],
                                            op=OP.mult)
                    nc.vector.tensor_reduce(
                        out=s_all[:, off:off + w],
                        in_=pr[:, :w * ADIM].rearrange("p (t d) -> p t d",
                                                       d=ADIM),
                        axis=AX.X, op=OP.add)
            nc.vector.tensor_copy(out=s16[:], in_=s_all[:])

            # ---- Phases 2+3 (interleaved emission) ------------------------
            # proj_all holds the unscaled projection for the whole shard so
            # the h(e) pipeline never waits on S; the S-scale is a final
            # sweep once the segment-sum has landed.
            proj_all = ppool.tile([6, EP], F16, tag="proj_all")
            supers = [(st0, min(SUP, EP - st0)) for st0 in range(0, EP, SUP)]
            B = 8
            with tc.tile_pool(name="ph2psum", bufs=1, space="PSUM") as sp, \
                 tc.tile_pool(name="ph2", bufs=6) as p2, \
                 tc.tile_pool(name="pm", bufs=2, space="PSUM") as pmp, \
                 tc.tile_pool(name="st", bufs=2, space="PSUM") as stp, \
                 tc.tile_pool(name="te", bufs=1, space="PSUM") as tep, \
                 tc.tile_pool(name="pj", bufs=1, space="PSUM") as pjp, \
                 tc.tile_pool(name="ph3", bufs=2) as p3:
                S_ps = sp.tile([64, PC], F32, tag="S_ps")
                # One accumulation group per PSUM bank: a zeroing matmul
                # (start=True) covers the bank, scatter matmuls accumulate
                # (start=False), the last generic matmul carries stop=True.
                z1 = p2.tile([1, 64], F16, tag="z1")
                nc.gpsimd.memset(z1[:], 0)
                nc.tensor.matmul(S_ps[0:64, 0:512], z1[:], io64_t[0:1, :],
                                 start=True, stop=False)
                nc.tensor.matmul(S_ps[0:64, 512:PC], z1[:],
                                 io64_t[0:1, 0:PC - 512],
                                 start=True, stop=False)

                def scatter_group(grp):
                    # Primary tile t covers local edges [64t, 64t+64); with
                    # el = c*64 + p this is psum column t, partitions 0:64.
                    for i in range(B):
                        t = grp * B + i
                        pone = p2.tile([128, 64], F16, tag="pone")
                        nc.vector.tensor_scalar(
                            out=pone[:], in0=io64_t[:, 0:64],
                            scalar1=r_t[:, t:t + 1],
                            scalar2=s_all[:, t:t + 1],
                            op0=OP.is_equal, op1=OP.mult)
                        nc.tensor.matmul(S_ps[0:64, t:t + 1],
                                         pone[:], ones_t[:],
                                         start=False, stop=False)

                def edge_super(st0, wd):
                    mt = p3.tile([128, SUP], F16, tag="mt")
                    nc.sync.dma_start(out=mt[:, :wd], in_=mT_d[:, st0:st0 + wd])
                    tl = p3.tile([12, SUP], F16, tag="tl")
                    nc.sync.dma_start(out=tl[:, :wd],
                                      in_=tail_d[:, st0:st0 + wd])
                    et = p3.tile([6, SUP], F16, tag="et")
                    nc.sync.dma_start(out=et[:, :wd],
                                      in_=tail_d[6:12, st0:st0 + wd])
                    for lo in range(0, wd, BLK):
                        sl = slice(lo, lo + BLK)
                        pm0 = pmp.tile([128, BLK], F32, tag="pm0")
                        nc.tensor.matmul(pm0[:], wm00_t[:], mt[:, sl],
                                         start=True, stop=False)
                        nc.tensor.matmul(pm0[:], wmk1_t[:], tl[0:6, sl],
                                         start=False, stop=True)
                        stg = stp.tile([38, BLK], F32, tag="stg")
                        nc.tensor.matmul(stg[:], wsa_t[:], mt[:, sl],
                                         start=True, stop=False)
                        nc.tensor.matmul(stg[:], wsb_t[:], tl[0:12, sl],
                                         start=False, stop=True)
                        te0 = tep.tile([128, BLK], F32, tag="te0")
                        nc.tensor.matmul(te0[:], we0_t[:], et[:, sl],
                                         start=True, stop=True)
                        m0 = p3.tile([128, BLK], F16, tag="m0")
                        nc.scalar.activation(out=m0[:], in_=pm0[:],
                                             func=silu_f, bias=b0_t[:, 0:1])
                        m1 = p3.tile([6, BLK], F16, tag="m1")
                        nc.scalar.activation(out=m1[:], in_=stg[0:6, :],
                                             func=silu_f, bias=b1_t[:, 0:1])
                        t0 = p3.tile([128, BLK], F16, tag="t0")
                        nc.scalar.activation(out=t0[:], in_=te0[:],
                                             func=ACTF.Copy)
                        me0 = p3.tile([128, BLK], F16, tag="me0")
                        nc.gpsimd.tensor_tensor(out=me0[:], in0=m0[:],
                                                in1=t0[:], op=OP.mult)
                        me1 = p3.tile([6, BLK], F16, tag="me1")
                        nc.vector.tensor_tensor(out=me1[:], in0=m1[:],
                                                in1=stg[32:38, :], op=OP.mult)
                        pj = pjp.tile([6, BLK], F32, tag="pj")
                        nc.tensor.matmul(pj[:], fw0_t[:], me0[:],
                                         start=True, stop=False)
                        nc.tensor.matmul(pj[:], fw1_t[:], me1[:],
                                         start=False, stop=True)
                        nc.scalar.activation(
                            out=proj_all[:, st0 + lo:st0 + lo + BLK],
                            in_=pj[:], func=ACTF.Copy)

                si = 0
                for grp in range(NT // B):
                    scatter_group(grp)
                    if grp % 4 == 3 and si < len(supers):
                        edge_super(*supers[si])
                        si += 1
                while si < len(supers):
                    edge_super(*supers[si])
                    si += 1

                for g in range(NG):
                    pg = p2.tile([128, 64], F16, tag="pg")
                    nc.vector.scalar_tensor_tensor(
                        out=pg[:], in0=io64_t[:, 0:64],
                        scalar=r_t[:, NT + g:NT + g + 1],
                        in1=s16[:, NT + g:NT + g + 1].to_broadcast([128, 64]),
                        op0=OP.is_equal, op1=OP.mult)
                    cg = p2.tile([128, PC], F16, tag="cgt")
                    nc.vector.tensor_scalar(
                        out=cg[:], in0=io392_t[:],
                        scalar1=cg_t[:, g:g + 1], scalar2=None,
                        op0=OP.is_equal)
                    nc.tensor.matmul(S_ps[0:64, 0:512], pg[:], cg[:, 0:512],
                                     start=False, stop=(g == NG - 1))
                    nc.tensor.matmul(S_ps[0:64, 512:PC], pg[:],
                                     cg[:, 512:PC],
                                     start=False, stop=(g == NG - 1))
                nc.scalar.activation(out=S_sb[:], in_=S_ps[:], func=ACTF.Copy)
                # Transpose S [p, c] -> [c, p] so DRAM flat order == el order.
                for q in range(8):
                    T_ps = pjp.tile([98, 64], F32, tag="pj")
                    nc.tensor.transpose(out=T_ps[:],
                                        in_=S_sb[0:64, 98 * q:98 * (q + 1)],
                                        identity=ident_t[0:64, 0:64])
                    nc.scalar.activation(out=S_sbT[:, 64 * q:64 * (q + 1)],
                                         in_=T_ps[:], func=ACTF.Copy)
                nc.sync.dma_start(
                    out=S_dramT[:].rearrange("(q r) p -> r q p", q=8),
                    in_=S_sbT[:].rearrange("r (q p) -> r q p", q=8))
                S_flat = S_dramT[:].rearrange("(o c) p -> o (c p)", o=1)

                # Final S-scale sweep over the whole shard.
                for st0, wd in supers:
                    s6 = p3.tile([6, SUP], F16, tag="s6")
                    nc.sync.dma_start(out=s6[:, :wd],
                                      in_=S_flat[0:1, st0:st0 + wd]
                                          .to_broadcast([6, wd]))
                    ob = p3.tile([6, SUP], F16, tag="ob")
                    nc.vector.tensor_tensor(out=ob[:, :wd],
                                            in0=proj_all[:, st0:st0 + wd],
                                            in1=s6[:, :wd], op=OP.mult)
                    nc.sync.dma_start(out=out_d[:, st0:st0 + wd],
                                      in_=ob[:, :wd])

    nc.compile()
    return nc


def _get_program():
    global _PROG
    if _PROG is None:
        _PROG = _build_program()
    return _PROG


def _f16(x):
    return np.ascontiguousarray(x, dtype=np.float16)


def _prep_inputs(m_ji, e_rbf, a_sbf, kj_idx, W_m, b_m, W_e, W_a, final_w):
    m_ji = np.asarray(m_ji, dtype=np.float32)
    e_rbf = np.asarray(e_rbf, dtype=np.float32)
    a_sbf = np.asarray(a_sbf, dtype=np.float32)
    kj = np.asarray(kj_idx).astype(np.int64).ravel()
    W_m = np.asarray(W_m, dtype=np.float32)
    b_m = np.asarray(b_m, dtype=np.float32).ravel()
    W_e = np.asarray(W_e, dtype=np.float32)
    W_a = np.asarray(W_a, dtype=np.float32)
    fw = np.asarray(final_w, dtype=np.float32)

    # shared weight-derived inputs
    WmT = W_m.T  # [c_in, c_out]
    WeT = W_e.T  # [6?? no: W_e is [CAT, NRBF]] -> W_e.T is [NRBF, CAT]
    fwT = fw.T   # [CAT, NRBF]
    wsa = np.zeros((128, 38), np.float32)
    wsa[:, 0:6] = WmT[:128, 128:134]
    wsb = np.zeros((12, 38), np.float32)
    wsb[0:6, 0:6] = WmT[128:134, 128:134]
    # te[c_out, e] = sum_j W_e[c_out, j] * e_rbf[e, j]; lhsT[k=j, m=c_out]
    # = W_e.T[j, c_out] = WeT[j, c_out] with WeT = W_e.T  ([NRBF, CAT])
    # te tail lands on psum partitions 32:38 (32-aligned engine reads).
    wsb[6:12, 32:38] = WeT[:, 128:134]
    w_sum = W_a.sum(axis=0)  # [42]

    shared = {
        "Wm00": _f16(WmT[:128, :128]),
        "WmK1M0": _f16(WmT[128:134, :128]),
        "WstragA": _f16(wsa),
        "WstragB": _f16(wsb),
        "WeT0": _f16(WeT[:, :128]),
        "fw0": _f16(fwT[:128, :]),
        "fw1": _f16(fwT[128:134, :]),
        "b0": np.ascontiguousarray(b_m[:128, None], np.float32),
        "b1": np.ascontiguousarray(b_m[128:134, None], np.float32),
        "wsum_rep": _f16(np.tile(w_sum, (128, PH1_CH))),
        "iota8x64": _f16(np.tile(np.arange(64), (128, 8))),
        "iota128": _f16(np.tile(np.arange(128), (128, 1))),
        "iota784": _f16(np.tile(np.arange(PC), (128, 1))),
        "ones128": _f16(np.ones((128, 1))),
        "ident128": np.eye(128, dtype=np.float32),
    }

    order = np.argsort(kj, kind="stable")
    kj_s = kj[order]
    a_s = a_sbf[order]
    bounds = np.searchsorted(kj_s, np.arange(NCORES + 1) * ESH)

    in_maps = []
    for i in range(NCORES):
        lo, hi = int(bounds[i]), int(bounds[i + 1])
        el = kj_s[lo:hi] - i * ESH          # sorted ascending in [0, ESH)
        av = a_s[lo:hi]
        n = el.shape[0]
        t_of = el // 64
        starts = np.searchsorted(t_of, np.arange(NT))
        rank = np.arange(n) - starts[t_of]
        prim = rank < 128
        r16 = np.zeros((128, NSLOT), np.float32)
        a_arr = np.zeros((128, NSLOT, ADIM), np.float16)
        cg16 = np.zeros((128, NG), np.float32)
        r16[rank[prim], t_of[prim]] = (el % 64)[prim]
        a_arr[rank[prim], t_of[prim], :] = av[prim]
        n_sp = int((~prim).sum())
        if n_sp > NG * 128:
            raise RuntimeError(f"core {i}: {n_sp} spill angles > {NG*128}")
        if n_sp:
            sp_el = el[~prim]
            sp_idx = np.arange(n_sp)
            rr, cc = sp_idx % 128, sp_idx // 128
            r16[rr, NT + cc] = sp_el % 64
            cg16[rr, cc] = sp_el // 64
            a_arr[rr, NT + cc, :] = av[~prim]

        msl = m_ji[i * ESH:(i + 1) * ESH]
        mT = np.zeros((CAT, EP), np.float16)
        mT[:, :ESH] = msl.T
        esl = e_rbf[i * ESH:(i + 1) * ESH]
        tailT = np.zeros((12, EP), np.float16)
        tailT[0:6, :ESH] = msl.T[128:134]
        tailT[6:12, :ESH] = esl.T

        im = dict(shared)
        im["a_arr"] = np.ascontiguousarray(a_arr.reshape(128, NSLOT * ADIM))
        im["r16"] = np.ascontiguousarray(r16)
        im["cg16"] = np.ascontiguousarray(cg16)
        im["mT0"] = np.ascontiguousarray(mT[:128])
        im["tailT"] = np.ascontiguousarray(tailT)
        in_maps.append(im)
    return in_maps


def kernel(m_ji, nbr_list, angle_list, e_rbf, a_sbf, kj_idx,
           W_m, b_m, W_e, W_a, final_w):
    global LAST_RESULT
    in_maps = _prep_inputs(m_ji, e_rbf, a_sbf, kj_idx, W_m, b_m, W_e, W_a,
                           final_w)
    nc = _get_program()
    res = run_bass_kernel_spmd(nc, in_maps, core_ids=list(range(NCORES)))
    LAST_RESULT = res
    out = np.empty((E, NRBF), np.float32)
    for i in range(NCORES):
        out[i * ESH:(i + 1) * ESH] = res.results[i]["out"][:, :ESH].T.astype(np.float32)
    return out


# revision 21
# speedup vs baseline: 1.1442x; 1.0601x over previous
"""Trainium2 Bass kernel for nn_DirectedMessage (gnn_message_passing).

Math: the reference's per-angle tensor m_and_e depends only on kj_idx[a], so
    final[e] = h(e) * S(e)
      h(e) = (silu(m_ji[e] @ W_m.T + b) * (e_rbf[e] @ W_e.T)) @ final_w.T   [E, 6]
      s[a] = a_sbf[a] . sum_r W_a[r]                                        [A]
      S(e) = segment_sum(s, kj_idx)[e]                                      [E]

Distribution (owner-computes): edges are sharded contiguously across the 8
cores; each angle is routed (on host, as part of sharding) to the core that
owns its kj edge, so no collective is needed.  Within a core, angles are
binned into fixed 64-edge windows (tile t covers local edges [64t, 64t+64));
the device computes s on-chip and performs the segment-sum with one small
PSUM-accumulating matmul per tile (lhsT = constant ones column, rhs = a
one-hot-times-s matrix built on the vector engine).  Overflow angles (>128
in one window) go through 4 generic full-width scatter tiles.
"""

import sys
import types

sys.path.insert(0, "/opt/trn_rl_repo")

# Optional NTFF trace hook (lets BASS_TRACE=1 capture hardware profiles).
try:  # pragma: no cover
    import trn_agent_boot.trn_boot as _tb

    if "antenv.axon_hooks" not in sys.modules:
        _hook = _tb._ntff_profile_via_ctypes("/opt/axon/libaxon_pjrt.so")
        _m = types.ModuleType("antenv.axon_hooks")
        _m.get_axon_ntff_profile_hook = lambda: _hook
        sys.modules["antenv.axon_hooks"] = _m
except Exception:
    pass

import os

import numpy as np

import concourse.bacc as bacc
import concourse.mybir as mybir
import concourse.tile as tile
from concourse.bass_utils import run_bass_kernel_spmd

F16 = mybir.dt.float16
F32 = mybir.dt.float32
OP = mybir.AluOpType
ACTF = mybir.ActivationFunctionType
AX = mybir.AxisListType

E = 400000
A = 600000
CAT = 134
NRBF = 6
ADIM = 42
NCORES = 8
ESH = E // NCORES          # 50000 edges per core
PW = 392                   # legacy name; see NT/PC below
EP = 50176                 # padded edges per core
PC = 784                   # S columns; e_local = c*64 + p, p in [0,64)
NT = EP // 64              # 784 primary scatter tiles (64-edge windows)
NG = 4                     # generic (overflow) scatter tiles
NSLOT = NT + NG            # 788 angle slot columns
SUP = 2048                 # edge super-block (columns per DMA)
BLK = 512                  # matmul moving width
PH1_CH = 16                # angle slot columns per phase-1 chunk

_PROG = None
LAST_RESULT = None


def _build_program():
    # CoreSim has no Silu; tests can force Sigmoid to validate dataflow.
    silu_f = (ACTF.Sigmoid if os.environ.get("KERNEL_SIM_ACT") == "sigmoid"
              else ACTF.Silu)
    nc = bacc.Bacc("TRN2", target_bir_lowering=False, debug=False,
                   num_devices=NCORES)

    a_d = nc.dram_tensor("a_arr", [128, NSLOT * ADIM], F16, kind="ExternalInput")
    r_d = nc.dram_tensor("r16", [128, NSLOT], F16, kind="ExternalInput")
    cg_d = nc.dram_tensor("cg16", [128, NG], F32, kind="ExternalInput")
    mT_d = nc.dram_tensor("mT0", [128, EP], F16, kind="ExternalInput")
    tail_d = nc.dram_tensor("tailT", [12, EP], F16, kind="ExternalInput")
    wm00_d = nc.dram_tensor("Wm00", [128, 128], F16, kind="ExternalInput")
    wmk1_d = nc.dram_tensor("WmK1M0", [6, 128], F16, kind="ExternalInput")
    wsa_d = nc.dram_tensor("WstragA", [128, 38], F16, kind="ExternalInput")
    wsb_d = nc.dram_tensor("WstragB", [12, 38], F16, kind="ExternalInput")
    we0_d = nc.dram_tensor("WeT0", [6, 128], F16, kind="ExternalInput")
    fw0_d = nc.dram_tensor("fw0", [128, 6], F16, kind="ExternalInput")
    fw1_d = nc.dram_tensor("fw1", [6, 6], F16, kind="ExternalInput")
    b0_d = nc.dram_tensor("b0", [128, 1], F32, kind="ExternalInput")
    b1_d = nc.dram_tensor("b1", [6, 1], F32, kind="ExternalInput")
    wsum_d = nc.dram_tensor("wsum_rep", [128, PH1_CH * ADIM], F16,
                            kind="ExternalInput")
    io64_d = nc.dram_tensor("iota8x64", [128, 512], F16, kind="ExternalInput")
    io64B_d = nc.dram_tensor("iota16x64", [128, 1024], F16, kind="ExternalInput")
    io128_d = nc.dram_tensor("iota128", [128, 128], F16, kind="ExternalInput")
    io392_d = nc.dram_tensor("iota784", [128, PC], F16, kind="ExternalInput")
    ones_d = nc.dram_tensor("ones128", [128, 1], F16, kind="ExternalInput")
    ident_d = nc.dram_tensor("ident128", [128, 128], F32, kind="ExternalInput")
    out_d = nc.dram_tensor("out", [6, EP], F16, kind="ExternalOutput")

    with tile.TileContext(nc) as tc:
        with tc.tile_pool(name="const", bufs=1) as cpool, \
             tc.tile_pool(name="dram", bufs=1, space="DRAM") as dpool, \
             tc.tile_pool(name="persist", bufs=1) as ppool:

            def cload(dram, shape, dtype=F16, tag=None):
                t = cpool.tile(shape, dtype, tag=tag or dram.name)
                nc.sync.dma_start(out=t[:], in_=dram[:])
                return t

            r_t = cload(r_d, [128, NSLOT])
            cg_t = cload(cg_d, [128, NG], F32)
            wm00_t = cload(wm00_d, [128, 128])
            wmk1_t = cload(wmk1_d, [6, 128])
            wsa_t = cload(wsa_d, [128, 38])
            wsb_t = cload(wsb_d, [12, 38])
            we0_t = cload(we0_d, [6, 128])
            fw0_t = cload(fw0_d, [128, 6])
            fw1_t = cload(fw1_d, [6, 6])
            b0_t = cload(b0_d, [128, 1], F32)
            b1_t = cload(b1_d, [6, 1], F32)
            wsum_t = cload(wsum_d, [128, PH1_CH * ADIM])
            io64_t = cload(io64_d, [128, 512])
            io64B_t = cload(io64B_d, [128, 1024])
            io128_t = cload(io128_d, [128, 128])
            io392_t = cload(io392_d, [128, PC])
            ones_t = cload(ones_d, [128, 1])
            ident_t = cload(ident_d, [128, 128], F32)

            s16 = ppool.tile([128, NSLOT], F16, tag="s16")
            S_sb = ppool.tile([64, PC], F32, tag="S_sb")
            S_sbT = ppool.tile([98, 512], F16, tag="S_sbT")
            S_dramT = dpool.tile([PC, 64], F16, tag="S_dramT")

            # ---- Phase 1: s[slot] = a_sbf[slot] . w_sum  -------------------
            with tc.tile_pool(name="ph1", bufs=3) as p1:
                for off in range(0, NSLOT, PH1_CH):
                    w = min(PH1_CH, NSLOT - off)
                    at = p1.tile([128, PH1_CH * ADIM], F16, tag="at")
                    nc.sync.dma_start(out=at[:, :w * ADIM],
                                      in_=a_d[:, off * ADIM:(off + w) * ADIM])
                    pr = p1.tile([128, PH1_CH * ADIM], F16, tag="pr")
                    nc.gpsimd.tensor_tensor(out=pr[:, :w * ADIM],
                                            in0=at[:, :w * ADIM],
                                            in1=wsum_t[:, :w * ADIM],
                                            op=OP.mult)
                    with nc.allow_low_precision("s fits fp16"):
                        nc.vector.tensor_reduce(
                            out=s16[:, off:off + w],
                            in_=pr[:, :w * ADIM].rearrange("p (t d) -> p t d",
                                                           d=ADIM),
                            axis=AX.X, op=OP.add)

            # ---- Phases 2+3 (interleaved emission) ------------------------
            # proj_all holds the unscaled projection for the whole shard so
            # the h(e) pipeline never waits on S; the S-scale is a final
            # sweep once the segment-sum has landed.
            proj_all = ppool.tile([6, EP], F16, tag="proj_all")
            supers = [(st0, min(SUP, EP - st0)) for st0 in range(0, EP, SUP)]
            B = 16
            with tc.tile_pool(name="ph2psum", bufs=1, space="PSUM") as sp, \
                 tc.tile_pool(name="ph2", bufs=6) as p2, \
                 tc.tile_pool(name="pm", bufs=2, space="PSUM") as pmp, \
                 tc.tile_pool(name="st", bufs=2, space="PSUM") as stp, \
                 tc.tile_pool(name="te", bufs=1, space="PSUM") as tep, \
                 tc.tile_pool(name="pj", bufs=1, space="PSUM") as pjp, \
                 tc.tile_pool(name="ph3", bufs=2) as p3:
                S_ps = sp.tile([64, PC], F32, tag="S_ps")
                # One accumulation group per PSUM bank: a zeroing matmul
                # (start=True) covers the bank, scatter matmuls accumulate
                # (start=False), the last generic matmul carries stop=True.
                z1 = p2.tile([1, 64], F16, tag="z1")
                nc.gpsimd.memset(z1[:], 0)
                nc.tensor.matmul(S_ps[0:64, 0:512], z1[:], io64_t[0:1, :],
                                 start=True, stop=False)
                nc.tensor.matmul(S_ps[0:64, 512:PC], z1[:],
                                 io64_t[0:1, 0:PC - 512],
                                 start=True, stop=False)

                def scatter_group(grp):
                    # Primary tile t covers local edges [64t, 64t+64); with
                    # el = c*64 + p this is psum column t, partitions 0:64.
                    # The eq tile is an unscaled one-hot; the multiply by s
                    # happens inside the matmul (rhs = s column).
                    eqB = p2.tile([128, B * 64], F16, tag="eqB")
                    nc.vector.tensor_tensor(
                        out=eqB[:].rearrange("p (t j) -> p t j", j=64),
                        in0=io64B_t[:].rearrange("p (t j) -> p t j", j=64),
                        in1=r_t[:, grp * B:(grp + 1) * B]
                            .to_broadcast([128, B, 64]),
                        op=OP.is_equal)
                    for i in range(B):
                        t = grp * B + i
                        nc.tensor.matmul(S_ps[0:64, t:t + 1],
                                         eqB[:, i * 64:(i + 1) * 64],
                                         s16[:, t:t + 1],
                                         start=False, stop=False)

                def edge_super(st0, wd):
                    mt = p3.tile([128, SUP], F16, tag="mt")
                    nc.sync.dma_start(out=mt[:, :wd], in_=mT_d[:, st0:st0 + wd])
                    tl = p3.tile([12, SUP], F16, tag="tl")
                    nc.sync.dma_start(out=tl[:, :wd],
                                      in_=tail_d[:, st0:st0 + wd])
                    et = p3.tile([6, SUP], F16, tag="et")
                    nc.sync.dma_start(out=et[:, :wd],
                                      in_=tail_d[6:12, st0:st0 + wd])
                    for lo in range(0, wd, BLK):
                        sl = slice(lo, lo + BLK)
                        pm0 = pmp.tile([128, BLK], F32, tag="pm0")
                        nc.tensor.matmul(pm0[:], wm00_t[:], mt[:, sl],
                                         start=True, stop=False)
                        nc.tensor.matmul(pm0[:], wmk1_t[:], tl[0:6, sl],
                                         start=False, stop=True)
                        stg = stp.tile([38, BLK], F32, tag="stg")
                        nc.tensor.matmul(stg[:], wsa_t[:], mt[:, sl],
                                         start=True, stop=False)
                        nc.tensor.matmul(stg[:], wsb_t[:], tl[0:12, sl],
                                         start=False, stop=True)
                        te0 = tep.tile([128, BLK], F32, tag="te0")
                        nc.tensor.matmul(te0[:], we0_t[:], et[:, sl],
                                         start=True, stop=True)
                        m0 = p3.tile([128, BLK], F16, tag="m0")
                        nc.scalar.activation(out=m0[:], in_=pm0[:],
                                             func=silu_f, bias=b0_t[:, 0:1])
                        m1 = p3.tile([6, BLK], F16, tag="m1")
                        nc.scalar.activation(out=m1[:], in_=stg[0:6, :],
                                             func=silu_f, bias=b1_t[:, 0:1])
                        t0 = p3.tile([128, BLK], F16, tag="t0")
                        nc.scalar.activation(out=t0[:], in_=te0[:],
                                             func=ACTF.Copy)
                        me0 = p3.tile([128, BLK], F16, tag="me0")
                        nc.vector.tensor_tensor(out=me0[:], in0=m0[:],
                                                in1=t0[:], op=OP.mult)
                        me1 = p3.tile([6, BLK], F16, tag="me1")
                        nc.vector.tensor_tensor(out=me1[:], in0=m1[:],
                                                in1=stg[32:38, :], op=OP.mult)
                        pj = pjp.tile([6, BLK], F32, tag="pj")
                        nc.tensor.matmul(pj[:], fw0_t[:], me0[:],
                                         start=True, stop=False)
                        nc.tensor.matmul(pj[:], fw1_t[:], me1[:],
                                         start=False, stop=True)
                        nc.scalar.activation(
                            out=proj_all[:, st0 + lo:st0 + lo + BLK],
                            in_=pj[:], func=ACTF.Copy)

                si = 0
                for grp in range(NT // B):
                    scatter_group(grp)
                    if grp % 4 == 3 and si < len(supers):
                        edge_super(*supers[si])
                        si += 1
                while si < len(supers):
                    edge_super(*supers[si])
                    si += 1

                for g in range(NG):
                    pg = p2.tile([128, 64], F16, tag="pg")
                    nc.vector.scalar_tensor_tensor(
                        out=pg[:], in0=io64_t[:, 0:64],
                        scalar=r_t[:, NT + g:NT + g + 1],
                        in1=s16[:, NT + g:NT + g + 1].to_broadcast([128, 64]),
                        op0=OP.is_equal, op1=OP.mult)
                    cg = p2.tile([128, PC], F16, tag="cgt")
                    nc.vector.tensor_scalar(
                        out=cg[:], in0=io392_t[:],
                        scalar1=cg_t[:, g:g + 1], scalar2=None,
                        op0=OP.is_equal)
                    nc.tensor.matmul(S_ps[0:64, 0:512], pg[:], cg[:, 0:512],
                                     start=False, stop=(g == NG - 1))
                    nc.tensor.matmul(S_ps[0:64, 512:PC], pg[:],
                                     cg[:, 512:PC],
                                     start=False, stop=(g == NG - 1))
                nc.scalar.activation(out=S_sb[:], in_=S_ps[:], func=ACTF.Copy)
                # Transpose S [p, c] -> [c, p] so DRAM flat order == el order.
                for q in range(8):
                    T_ps = pjp.tile([98, 64], F32, tag="pj")
                    nc.tensor.transpose(out=T_ps[:],
                                        in_=S_sb[0:64, 98 * q:98 * (q + 1)],
                                        identity=ident_t[0:64, 0:64])
                    nc.scalar.activation(out=S_sbT[:, 64 * q:64 * (q + 1)],
                                         in_=T_ps[:], func=ACTF.Copy)
                nc.sync.dma_start(
                    out=S_dramT[:].rearrange("(q r) p -> r q p", q=8),
                    in_=S_sbT[:].rearrange("r (q p) -> r q p", q=8))
                S_flat = S_dramT[:].rearrange("(o c) p -> o (c p)", o=1)

                # Final S-scale sweep over the whole shard.
                for st0, wd in supers:
                    s6 = p3.tile([6, SUP], F16, tag="s6")
                    nc.sync.dma_start(out=s6[:, :wd],
                                      in_=S_flat[0:1, st0:st0 + wd]
                                          .to_broadcast([6, wd]))
                    ob = p3.tile([6, SUP], F16, tag="ob")
                    nc.vector.tensor_tensor(out=ob[:, :wd],
                                            in0=proj_all[:, st0:st0 + wd],
                                            in1=s6[:, :wd], op=OP.mult)
                    nc.sync.dma_start(out=out_d[:, st0:st0 + wd],
                                      in_=ob[:, :wd])

    nc.compile()
    return nc


def _get_program():
    global _PROG
    if _PROG is None:
        _PROG = _build_program()
    return _PROG


def _f16(x):
    return np.ascontiguousarray(x, dtype=np.float16)


def _prep_inputs(m_ji, e_rbf, a_sbf, kj_idx, W_m, b_m, W_e, W_a, final_w):
    m_ji = np.asarray(m_ji, dtype=np.float32)
    e_rbf = np.asarray(e_rbf, dtype=np.float32)
    a_sbf = np.asarray(a_sbf, dtype=np.float32)
    kj = np.asarray(kj_idx).astype(np.int64).ravel()
    W_m = np.asarray(W_m, dtype=np.float32)
    b_m = np.asarray(b_m, dtype=np.float32).ravel()
    W_e = np.asarray(W_e, dtype=np.float32)
    W_a = np.asarray(W_a, dtype=np.float32)
    fw = np.asarray(final_w, dtype=np.float32)

    # shared weight-derived inputs
    WmT = W_m.T  # [c_in, c_out]
    WeT = W_e.T  # [6?? no: W_e is [CAT, NRBF]] -> W_e.T is [NRBF, CAT]
    fwT = fw.T   # [CAT, NRBF]
    wsa = np.zeros((128, 38), np.float32)
    wsa[:, 0:6] = WmT[:128, 128:134]
    wsb = np.zeros((12, 38), np.float32)
    wsb[0:6, 0:6] = WmT[128:134, 128:134]
    # te[c_out, e] = sum_j W_e[c_out, j] * e_rbf[e, j]; lhsT[k=j, m=c_out]
    # = W_e.T[j, c_out] = WeT[j, c_out] with WeT = W_e.T  ([NRBF, CAT])
    # te tail lands on psum partitions 32:38 (32-aligned engine reads).
    wsb[6:12, 32:38] = WeT[:, 128:134]
    w_sum = W_a.sum(axis=0)  # [42]

    shared = {
        "Wm00": _f16(WmT[:128, :128]),
        "WmK1M0": _f16(WmT[128:134, :128]),
        "WstragA": _f16(wsa),
        "WstragB": _f16(wsb),
        "WeT0": _f16(WeT[:, :128]),
        "fw0": _f16(fwT[:128, :]),
        "fw1": _f16(fwT[128:134, :]),
        "b0": np.ascontiguousarray(b_m[:128, None], np.float32),
        "b1": np.ascontiguousarray(b_m[128:134, None], np.float32),
        "wsum_rep": _f16(np.tile(w_sum, (128, PH1_CH))),
        "iota8x64": _f16(np.tile(np.arange(64), (128, 8))),
        "iota16x64": _f16(np.tile(np.arange(64), (128, 16))),
        "iota128": _f16(np.tile(np.arange(128), (128, 1))),
        "iota784": _f16(np.tile(np.arange(PC), (128, 1))),
        "ones128": _f16(np.ones((128, 1))),
        "ident128": np.eye(128, dtype=np.float32),
    }

    order = np.argsort(kj, kind="stable")
    kj_s = kj[order]
    a_s = a_sbf[order]
    bounds = np.searchsorted(kj_s, np.arange(NCORES + 1) * ESH)

    in_maps = []
    for i in range(NCORES):
        lo, hi = int(bounds[i]), int(bounds[i + 1])
        el = kj_s[lo:hi] - i * ESH          # sorted ascending in [0, ESH)
        av = a_s[lo:hi]
        n = el.shape[0]
        t_of = el // 64
        starts = np.searchsorted(t_of, np.arange(NT))
        rank = np.arange(n) - starts[t_of]
        prim = rank < 128
        r16 = np.zeros((128, NSLOT), np.float16)
        a_arr = np.zeros((128, NSLOT, ADIM), np.float16)
        cg16 = np.zeros((128, NG), np.float32)
        r16[rank[prim], t_of[prim]] = (el % 64)[prim]
        a_arr[rank[prim], t_of[prim], :] = av[prim]
        n_sp = int((~prim).sum())
        if n_sp > NG * 128:
            raise RuntimeError(f"core {i}: {n_sp} spill angles > {NG*128}")
        if n_sp:
            sp_el = el[~prim]
            sp_idx = np.arange(n_sp)
            rr, cc = sp_idx % 128, sp_idx // 128
            r16[rr, NT + cc] = sp_el % 64
            cg16[rr, cc] = sp_el // 64
            a_arr[rr, NT + cc, :] = av[~prim]

        msl = m_ji[i * ESH:(i + 1) * ESH]
        mT = np.zeros((CAT, EP), np.float16)
        mT[:, :ESH] = msl.T
        esl = e_rbf[i * ESH:(i + 1) * ESH]
        tailT = np.zeros((12, EP), np.float16)
        tailT[0:6, :ESH] = msl.T[128:134]
        tailT[6:12, :ESH] = esl.T

        im = dict(shared)
        im["a_arr"] = np.ascontiguousarray(a_arr.reshape(128, NSLOT * ADIM))
        im["r16"] = np.ascontiguousarray(r16)
        im["cg16"] = np.ascontiguousarray(cg16)
        im["mT0"] = np.ascontiguousarray(mT[:128])
        im["tailT"] = np.ascontiguousarray(tailT)
        in_maps.append(im)
    return in_maps


def kernel(m_ji, nbr_list, angle_list, e_rbf, a_sbf, kj_idx,
           W_m, b_m, W_e, W_a, final_w):
    global LAST_RESULT
    in_maps = _prep_inputs(m_ji, e_rbf, a_sbf, kj_idx, W_m, b_m, W_e, W_a,
                           final_w)
    nc = _get_program()
    res = run_bass_kernel_spmd(nc, in_maps, core_ids=list(range(NCORES)))
    LAST_RESULT = res
    out = np.empty((E, NRBF), np.float32)
    for i in range(NCORES):
        out[i * ESH:(i + 1) * ESH] = res.results[i]["out"][:, :ESH].T.astype(np.float32)
    return out


# revision 22
# speedup vs baseline: 1.1745x; 1.0265x over previous
"""Trainium2 Bass kernel for nn_DirectedMessage (gnn_message_passing).

Math: the reference's per-angle tensor m_and_e depends only on kj_idx[a], so
    final[e] = h(e) * S(e)
      h(e) = (silu(m_ji[e] @ W_m.T + b) * (e_rbf[e] @ W_e.T)) @ final_w.T   [E, 6]
      s[a] = a_sbf[a] . sum_r W_a[r]                                        [A]
      S(e) = segment_sum(s, kj_idx)[e]                                      [E]

Distribution (owner-computes): edges are sharded contiguously across the 8
cores; each angle is routed (on host, as part of sharding) to the core that
owns its kj edge, so no collective is needed.  Within a core, angles are
binned into fixed 64-edge windows (tile t covers local edges [64t, 64t+64));
the device computes s on-chip and performs the segment-sum with one small
PSUM-accumulating matmul per tile (lhsT = constant ones column, rhs = a
one-hot-times-s matrix built on the vector engine).  Overflow angles (>128
in one window) go through 4 generic full-width scatter tiles.
"""

import sys
import types

sys.path.insert(0, "/opt/trn_rl_repo")

# Optional NTFF trace hook (lets BASS_TRACE=1 capture hardware profiles).
try:  # pragma: no cover
    import trn_agent_boot.trn_boot as _tb

    if "antenv.axon_hooks" not in sys.modules:
        _hook = _tb._ntff_profile_via_ctypes("/opt/axon/libaxon_pjrt.so")
        _m = types.ModuleType("antenv.axon_hooks")
        _m.get_axon_ntff_profile_hook = lambda: _hook
        sys.modules["antenv.axon_hooks"] = _m
except Exception:
    pass

import os

import numpy as np

import concourse.bacc as bacc
import concourse.mybir as mybir
import concourse.tile as tile
from concourse.bass_utils import run_bass_kernel_spmd

F16 = mybir.dt.float16
BF16 = mybir.dt.bfloat16
F32 = mybir.dt.float32
OP = mybir.AluOpType
ACTF = mybir.ActivationFunctionType
AX = mybir.AxisListType

E = 400000
A = 600000
CAT = 134
NRBF = 6
ADIM = 42
NCORES = 8
ESH = E // NCORES          # 50000 edges per core
PW = 392                   # legacy name; see NT/PC below
EP = 50176                 # padded edges per core
PC = 784                   # S columns; e_local = c*64 + p, p in [0,64)
NT = EP // 64              # 784 primary scatter tiles (64-edge windows)
NG = 4                     # generic (overflow) scatter tiles
NSLOT = NT + NG            # 788 angle slot columns
SUP = 2048                 # edge super-block (columns per DMA)
BLK = 512                  # matmul moving width
PH1_CH = 16                # angle slot columns per phase-1 chunk

_PROG = None
LAST_RESULT = None


def _build_program():
    # CoreSim has no Silu; tests can force Sigmoid to validate dataflow.
    silu_f = (ACTF.Sigmoid if os.environ.get("KERNEL_SIM_ACT") == "sigmoid"
              else ACTF.Silu)
    nc = bacc.Bacc("TRN2", target_bir_lowering=False, debug=False,
                   num_devices=NCORES)

    a_d = nc.dram_tensor("a_arr", [128, NSLOT * ADIM], F16, kind="ExternalInput")
    r_d = nc.dram_tensor("r16", [128, NSLOT], F16, kind="ExternalInput")
    cg_d = nc.dram_tensor("cg16", [128, NG], F32, kind="ExternalInput")
    mT_d = nc.dram_tensor("mT0", [128, EP], BF16, kind="ExternalInput")
    tail_d = nc.dram_tensor("tailT", [12, EP], BF16, kind="ExternalInput")
    wm00_d = nc.dram_tensor("Wm00", [128, 128], BF16, kind="ExternalInput")
    wmk1_d = nc.dram_tensor("WmK1M0", [6, 128], BF16, kind="ExternalInput")
    wsa_d = nc.dram_tensor("WstragA", [128, 38], BF16, kind="ExternalInput")
    wsb_d = nc.dram_tensor("WstragB", [12, 38], BF16, kind="ExternalInput")
    we0_d = nc.dram_tensor("WeT0", [6, 128], BF16, kind="ExternalInput")
    fw0_d = nc.dram_tensor("fw0", [128, 6], BF16, kind="ExternalInput")
    fw1_d = nc.dram_tensor("fw1", [6, 6], BF16, kind="ExternalInput")
    b0_d = nc.dram_tensor("b0", [128, 1], F32, kind="ExternalInput")
    b1_d = nc.dram_tensor("b1", [6, 1], F32, kind="ExternalInput")
    wsum_d = nc.dram_tensor("wsum_rep", [128, PH1_CH * ADIM], F16,
                            kind="ExternalInput")
    io64_d = nc.dram_tensor("iota8x64", [128, 512], F16, kind="ExternalInput")
    io64B_d = nc.dram_tensor("iota16x64", [128, 1024], F16, kind="ExternalInput")
    io128_d = nc.dram_tensor("iota128", [128, 128], F16, kind="ExternalInput")
    io392_d = nc.dram_tensor("iota784", [128, PC], F16, kind="ExternalInput")
    ones_d = nc.dram_tensor("ones128", [128, 1], F16, kind="ExternalInput")
    ident_d = nc.dram_tensor("ident128", [128, 128], F32, kind="ExternalInput")
    out_d = nc.dram_tensor("out", [6, EP], F16, kind="ExternalOutput")

    with tile.TileContext(nc) as tc:
        with tc.tile_pool(name="const", bufs=1) as cpool, \
             tc.tile_pool(name="dram", bufs=1, space="DRAM") as dpool, \
             tc.tile_pool(name="persist", bufs=1) as ppool:

            def cload(dram, shape, dtype=F16, tag=None):
                t = cpool.tile(shape, dtype, tag=tag or dram.name)
                nc.sync.dma_start(out=t[:], in_=dram[:])
                return t

            r_t = cload(r_d, [128, NSLOT])
            cg_t = cload(cg_d, [128, NG], F32)
            wm00_t = cload(wm00_d, [128, 128], BF16)
            wmk1_t = cload(wmk1_d, [6, 128], BF16)
            wsa_t = cload(wsa_d, [128, 38], BF16)
            wsb_t = cload(wsb_d, [12, 38], BF16)
            we0_t = cload(we0_d, [6, 128], BF16)
            fw0_t = cload(fw0_d, [128, 6], BF16)
            fw1_t = cload(fw1_d, [6, 6], BF16)
            b0_t = cload(b0_d, [128, 1], F32)
            b1_t = cload(b1_d, [6, 1], F32)
            wsum_t = cload(wsum_d, [128, PH1_CH * ADIM])
            io64_t = cload(io64_d, [128, 512])
            io64B_t = cload(io64B_d, [128, 1024])
            io128_t = cload(io128_d, [128, 128])
            io392_t = cload(io392_d, [128, PC])
            ones_t = cload(ones_d, [128, 1])
            ident_t = cload(ident_d, [128, 128], F32)

            s16 = ppool.tile([128, NSLOT], F16, tag="s16")
            S_sb = ppool.tile([64, PC], F32, tag="S_sb")
            S_sbT = ppool.tile([98, 512], F16, tag="S_sbT")
            S_dramT = dpool.tile([PC, 64], F16, tag="S_dramT")

            # ---- Phase 1: s[slot] = a_sbf[slot] . w_sum  -------------------
            with tc.tile_pool(name="ph1", bufs=3) as p1:
                for off in range(0, NSLOT, PH1_CH):
                    w = min(PH1_CH, NSLOT - off)
                    at = p1.tile([128, PH1_CH * ADIM], F16, tag="at")
                    nc.sync.dma_start(out=at[:, :w * ADIM],
                                      in_=a_d[:, off * ADIM:(off + w) * ADIM])
                    pr = p1.tile([128, PH1_CH * ADIM], F16, tag="pr")
                    nc.gpsimd.tensor_tensor(out=pr[:, :w * ADIM],
                                            in0=at[:, :w * ADIM],
                                            in1=wsum_t[:, :w * ADIM],
                                            op=OP.mult)
                    with nc.allow_low_precision("s fits fp16"):
                        nc.vector.tensor_reduce(
                            out=s16[:, off:off + w],
                            in_=pr[:, :w * ADIM].rearrange("p (t d) -> p t d",
                                                           d=ADIM),
                            axis=AX.X, op=OP.add)

            # ---- Phases 2+3 (interleaved emission) ------------------------
            # proj_all holds the unscaled projection for the whole shard so
            # the h(e) pipeline never waits on S; the S-scale is a final
            # sweep once the segment-sum has landed.
            proj_all = ppool.tile([6, EP], F16, tag="proj_all")
            supers = [(st0, min(SUP, EP - st0)) for st0 in range(0, EP, SUP)]
            B = 16
            with tc.tile_pool(name="ph2psum", bufs=1, space="PSUM") as sp, \
                 tc.tile_pool(name="ph2", bufs=6) as p2, \
                 tc.tile_pool(name="pm", bufs=2, space="PSUM") as pmp, \
                 tc.tile_pool(name="st", bufs=2, space="PSUM") as stp, \
                 tc.tile_pool(name="te", bufs=1, space="PSUM") as tep, \
                 tc.tile_pool(name="pj", bufs=1, space="PSUM") as pjp, \
                 tc.tile_pool(name="ph3", bufs=2) as p3:
                S_ps = sp.tile([64, PC], F32, tag="S_ps")
                # One accumulation group per PSUM bank: a zeroing matmul
                # (start=True) covers the bank, scatter matmuls accumulate
                # (start=False), the last generic matmul carries stop=True.
                z1 = p2.tile([1, 64], F16, tag="z1")
                nc.gpsimd.memset(z1[:], 0)
                nc.tensor.matmul(S_ps[0:64, 0:512], z1[:], io64_t[0:1, :],
                                 start=True, stop=False)
                nc.tensor.matmul(S_ps[0:64, 512:PC], z1[:],
                                 io64_t[0:1, 0:PC - 512],
                                 start=True, stop=False)

                def scatter_group(grp):
                    # Primary tile t covers local edges [64t, 64t+64); with
                    # el = c*64 + p this is psum column t, partitions 0:64.
                    # The eq tile is an unscaled one-hot; the multiply by s
                    # happens inside the matmul (rhs = s column).
                    eqB = p2.tile([128, B * 64], F16, tag="eqB")
                    nc.vector.tensor_tensor(
                        out=eqB[:].rearrange("p (t j) -> p t j", j=64),
                        in0=io64B_t[:].rearrange("p (t j) -> p t j", j=64),
                        in1=r_t[:, grp * B:(grp + 1) * B]
                            .to_broadcast([128, B, 64]),
                        op=OP.is_equal)
                    for i in range(B):
                        t = grp * B + i
                        nc.tensor.matmul(S_ps[0:64, t:t + 1],
                                         eqB[:, i * 64:(i + 1) * 64],
                                         s16[:, t:t + 1],
                                         start=False, stop=False)

                def edge_super(st0, wd):
                    mt = p3.tile([128, SUP], BF16, tag="mt")
                    nc.sync.dma_start(out=mt[:, :wd], in_=mT_d[:, st0:st0 + wd])
                    tl = p3.tile([12, SUP], BF16, tag="tl")
                    nc.sync.dma_start(out=tl[:, :wd],
                                      in_=tail_d[:, st0:st0 + wd])
                    et = p3.tile([6, SUP], BF16, tag="et")
                    nc.sync.dma_start(out=et[:, :wd],
                                      in_=tail_d[6:12, st0:st0 + wd])
                    for lo in range(0, wd, BLK):
                        sl = slice(lo, lo + BLK)
                        pm0 = pmp.tile([128, BLK], F32, tag="pm0")
                        nc.tensor.matmul(pm0[:], wm00_t[:], mt[:, sl],
                                         start=True, stop=False)
                        nc.tensor.matmul(pm0[:], wmk1_t[:], tl[0:6, sl],
                                         start=False, stop=True)
                        stg = stp.tile([38, BLK], F32, tag="stg")
                        nc.tensor.matmul(stg[:], wsa_t[:], mt[:, sl],
                                         start=True, stop=False)
                        nc.tensor.matmul(stg[:], wsb_t[:], tl[0:12, sl],
                                         start=False, stop=True)
                        te0 = tep.tile([128, BLK], F32, tag="te0")
                        nc.tensor.matmul(te0[:], we0_t[:], et[:, sl],
                                         start=True, stop=True)
                        m0 = p3.tile([128, BLK], BF16, tag="m0")
                        nc.scalar.activation(out=m0[:], in_=pm0[:],
                                             func=silu_f, bias=b0_t[:, 0:1])
                        m1 = p3.tile([6, BLK], BF16, tag="m1")
                        nc.scalar.activation(out=m1[:], in_=stg[0:6, :],
                                             func=silu_f, bias=b1_t[:, 0:1])
                        t0 = p3.tile([128, BLK], BF16, tag="t0")
                        nc.scalar.activation(out=t0[:], in_=te0[:],
                                             func=ACTF.Copy)
                        me0 = p3.tile([128, BLK], BF16, tag="me0")
                        nc.vector.tensor_tensor(out=me0[:], in0=m0[:],
                                                in1=t0[:], op=OP.mult)
                        me1 = p3.tile([6, BLK], BF16, tag="me1")
                        nc.vector.tensor_tensor(out=me1[:], in0=m1[:],
                                                in1=stg[32:38, :], op=OP.mult)
                        pj = pjp.tile([6, BLK], F32, tag="pj")
                        nc.tensor.matmul(pj[:], fw0_t[:], me0[:],
                                         start=True, stop=False)
                        nc.tensor.matmul(pj[:], fw1_t[:], me1[:],
                                         start=False, stop=True)
                        nc.scalar.activation(
                            out=proj_all[:, st0 + lo:st0 + lo + BLK],
                            in_=pj[:], func=ACTF.Copy)

                si = 0
                for grp in range(NT // B):
                    scatter_group(grp)
                    if grp % 4 == 3 and si < len(supers):
                        edge_super(*supers[si])
                        si += 1
                while si < len(supers):
                    edge_super(*supers[si])
                    si += 1

                for g in range(NG):
                    pg = p2.tile([128, 64], F16, tag="pg")
                    nc.vector.scalar_tensor_tensor(
                        out=pg[:], in0=io64_t[:, 0:64],
                        scalar=r_t[:, NT + g:NT + g + 1],
                        in1=s16[:, NT + g:NT + g + 1].to_broadcast([128, 64]),
                        op0=OP.is_equal, op1=OP.mult)
                    cg = p2.tile([128, PC], F16, tag="cgt")
                    nc.vector.tensor_scalar(
                        out=cg[:], in0=io392_t[:],
                        scalar1=cg_t[:, g:g + 1], scalar2=None,
                        op0=OP.is_equal)
                    nc.tensor.matmul(S_ps[0:64, 0:512], pg[:], cg[:, 0:512],
                                     start=False, stop=(g == NG - 1))
                    nc.tensor.matmul(S_ps[0:64, 512:PC], pg[:],
                                     cg[:, 512:PC],
                                     start=False, stop=(g == NG - 1))
                nc.scalar.activation(out=S_sb[:], in_=S_ps[:], func=ACTF.Copy)
                # Transpose S [p, c] -> [c, p] so DRAM flat order == el order.
                for q in range(8):
                    T_ps = pjp.tile([98, 64], F32, tag="pj")
                    nc.tensor.transpose(out=T_ps[:],
                                        in_=S_sb[0:64, 98 * q:98 * (q + 1)],
                                        identity=ident_t[0:64, 0:64])
                    nc.scalar.activation(out=S_sbT[:, 64 * q:64 * (q + 1)],
                                         in_=T_ps[:], func=ACTF.Copy)
                nc.sync.dma_start(
                    out=S_dramT[:].rearrange("(q r) p -> r q p", q=8),
                    in_=S_sbT[:].rearrange("r (q p) -> r q p", q=8))
                S_flat = S_dramT[:].rearrange("(o c) p -> o (c p)", o=1)

                # Final S-scale sweep over the whole shard.
                for st0, wd in supers:
                    s6 = p3.tile([6, SUP], F16, tag="s6")
                    nc.sync.dma_start(out=s6[:, :wd],
                                      in_=S_flat[0:1, st0:st0 + wd]
                                          .to_broadcast([6, wd]))
                    ob = p3.tile([6, SUP], F16, tag="ob")
                    nc.vector.tensor_tensor(out=ob[:, :wd],
                                            in0=proj_all[:, st0:st0 + wd],
                                            in1=s6[:, :wd], op=OP.mult)
                    nc.sync.dma_start(out=out_d[:, st0:st0 + wd],
                                      in_=ob[:, :wd])

    nc.compile()
    return nc


def _get_program():
    global _PROG
    if _PROG is None:
        _PROG = _build_program()
    return _PROG


import ml_dtypes


def _f16(x):
    return np.ascontiguousarray(x, dtype=np.float16)


def _bf16(x):
    return np.ascontiguousarray(np.asarray(x, np.float32).astype(ml_dtypes.bfloat16))


def _prep_inputs(m_ji, e_rbf, a_sbf, kj_idx, W_m, b_m, W_e, W_a, final_w):
    m_ji = np.asarray(m_ji, dtype=np.float32)
    e_rbf = np.asarray(e_rbf, dtype=np.float32)
    a_sbf = np.asarray(a_sbf, dtype=np.float32)
    kj = np.asarray(kj_idx).astype(np.int64).ravel()
    W_m = np.asarray(W_m, dtype=np.float32)
    b_m = np.asarray(b_m, dtype=np.float32).ravel()
    W_e = np.asarray(W_e, dtype=np.float32)
    W_a = np.asarray(W_a, dtype=np.float32)
    fw = np.asarray(final_w, dtype=np.float32)

    # shared weight-derived inputs
    WmT = W_m.T  # [c_in, c_out]
    WeT = W_e.T  # [6?? no: W_e is [CAT, NRBF]] -> W_e.T is [NRBF, CAT]
    fwT = fw.T   # [CAT, NRBF]
    wsa = np.zeros((128, 38), np.float32)
    wsa[:, 0:6] = WmT[:128, 128:134]
    wsb = np.zeros((12, 38), np.float32)
    wsb[0:6, 0:6] = WmT[128:134, 128:134]
    # te[c_out, e] = sum_j W_e[c_out, j] * e_rbf[e, j]; lhsT[k=j, m=c_out]
    # = W_e.T[j, c_out] = WeT[j, c_out] with WeT = W_e.T  ([NRBF, CAT])
    # te tail lands on psum partitions 32:38 (32-aligned engine reads).
    wsb[6:12, 32:38] = WeT[:, 128:134]
    w_sum = W_a.sum(axis=0)  # [42]

    shared = {
        "Wm00": _bf16(WmT[:128, :128]),
        "WmK1M0": _bf16(WmT[128:134, :128]),
        "WstragA": _bf16(wsa),
        "WstragB": _bf16(wsb),
        "WeT0": _bf16(WeT[:, :128]),
        "fw0": _bf16(fwT[:128, :]),
        "fw1": _bf16(fwT[128:134, :]),
        "b0": np.ascontiguousarray(b_m[:128, None], np.float32),
        "b1": np.ascontiguousarray(b_m[128:134, None], np.float32),
        "wsum_rep": _f16(np.tile(w_sum, (128, PH1_CH))),
        "iota8x64": _f16(np.tile(np.arange(64), (128, 8))),
        "iota16x64": _f16(np.tile(np.arange(64), (128, 16))),
        "iota128": _f16(np.tile(np.arange(128), (128, 1))),
        "iota784": _f16(np.tile(np.arange(PC), (128, 1))),
        "ones128": _f16(np.ones((128, 1))),
        "ident128": np.eye(128, dtype=np.float32),
    }

    order = np.argsort(kj, kind="stable")
    kj_s = kj[order]
    a_s = a_sbf[order]
    bounds = np.searchsorted(kj_s, np.arange(NCORES + 1) * ESH)

    in_maps = []
    for i in range(NCORES):
        lo, hi = int(bounds[i]), int(bounds[i + 1])
        el = kj_s[lo:hi] - i * ESH          # sorted ascending in [0, ESH)
        av = a_s[lo:hi]
        n = el.shape[0]
        t_of = el // 64
        starts = np.searchsorted(t_of, np.arange(NT))
        rank = np.arange(n) - starts[t_of]
        prim = rank < 128
        r16 = np.zeros((128, NSLOT), np.float16)
        a_arr = np.zeros((128, NSLOT, ADIM), np.float16)
        cg16 = np.zeros((128, NG), np.float32)
        r16[rank[prim], t_of[prim]] = (el % 64)[prim]
        a_arr[rank[prim], t_of[prim], :] = av[prim]
        n_sp = int((~prim).sum())
        if n_sp > NG * 128:
            raise RuntimeError(f"core {i}: {n_sp} spill angles > {NG*128}")
        if n_sp:
            sp_el = el[~prim]
            sp_idx = np.arange(n_sp)
            rr, cc = sp_idx % 128, sp_idx // 128
            r16[rr, NT + cc] = sp_el % 64
            cg16[rr, cc] = sp_el // 64
            a_arr[rr, NT + cc, :] = av[~prim]

        msl = m_ji[i * ESH:(i + 1) * ESH]
        mT = np.zeros((CAT, EP), ml_dtypes.bfloat16)
        mT[:, :ESH] = msl.T.astype(ml_dtypes.bfloat16)
        esl = e_rbf[i * ESH:(i + 1) * ESH]
        tailT = np.zeros((12, EP), ml_dtypes.bfloat16)
        tailT[0:6, :ESH] = msl.T[128:134].astype(ml_dtypes.bfloat16)
        tailT[6:12, :ESH] = esl.T.astype(ml_dtypes.bfloat16)

        im = dict(shared)
        im["a_arr"] = np.ascontiguousarray(a_arr.reshape(128, NSLOT * ADIM))
        im["r16"] = np.ascontiguousarray(r16)
        im["cg16"] = np.ascontiguousarray(cg16)
        im["mT0"] = np.ascontiguousarray(mT[:128])
        im["tailT"] = np.ascontiguousarray(tailT)
        in_maps.append(im)
    return in_maps


def kernel(m_ji, nbr_list, angle_list, e_rbf, a_sbf, kj_idx,
           W_m, b_m, W_e, W_a, final_w):
    global LAST_RESULT
    in_maps = _prep_inputs(m_ji, e_rbf, a_sbf, kj_idx, W_m, b_m, W_e, W_a,
                           final_w)
    nc = _get_program()
    res = run_bass_kernel_spmd(nc, in_maps, core_ids=list(range(NCORES)))
    LAST_RESULT = res
    out = np.empty((E, NRBF), np.float32)
    for i in range(NCORES):
        out[i * ESH:(i + 1) * ESH] = res.results[i]["out"][:, :ESH].T.astype(np.float32)
    return out


# revision 24
# speedup vs baseline: 1.1772x; 1.0023x over previous
"""Trainium2 Bass kernel for nn_DirectedMessage (gnn_message_passing).

Math: the reference's per-angle tensor m_and_e depends only on kj_idx[a], so
    final[e] = h(e) * S(e)
      h(e) = (silu(m_ji[e] @ W_m.T + b) * (e_rbf[e] @ W_e.T)) @ final_w.T   [E, 6]
      s[a] = a_sbf[a] . sum_r W_a[r]                                        [A]
      S(e) = segment_sum(s, kj_idx)[e]                                      [E]

Distribution (owner-computes): edges are sharded contiguously across the 8
cores; each angle is routed (on host, as part of sharding) to the core that
owns its kj edge, so no collective is needed.  Within a core, angles are
binned into fixed 64-edge windows (tile t covers local edges [64t, 64t+64));
the device computes s on-chip and performs the segment-sum with one small
PSUM-accumulating matmul per tile (lhsT = constant ones column, rhs = a
one-hot-times-s matrix built on the vector engine).  Overflow angles (>128
in one window) go through 4 generic full-width scatter tiles.
"""

import sys
import types

sys.path.insert(0, "/opt/trn_rl_repo")

# Optional NTFF trace hook (lets BASS_TRACE=1 capture hardware profiles).
try:  # pragma: no cover
    import trn_agent_boot.trn_boot as _tb

    if "antenv.axon_hooks" not in sys.modules:
        _hook = _tb._ntff_profile_via_ctypes("/opt/axon/libaxon_pjrt.so")
        _m = types.ModuleType("antenv.axon_hooks")
        _m.get_axon_ntff_profile_hook = lambda: _hook
        sys.modules["antenv.axon_hooks"] = _m
except Exception:
    pass

import os

import numpy as np

import concourse.bacc as bacc
import concourse.mybir as mybir
import concourse.tile as tile
from concourse.bass_utils import run_bass_kernel_spmd

F16 = mybir.dt.float16
BF16 = mybir.dt.bfloat16
F32 = mybir.dt.float32
OP = mybir.AluOpType
ACTF = mybir.ActivationFunctionType
AX = mybir.AxisListType

E = 400000
A = 600000
CAT = 134
NRBF = 6
ADIM = 42
NCORES = 8
ESH = E // NCORES          # 50000 edges per core
PW = 392                   # legacy name; see NT/PC below
EP = 50176                 # padded edges per core
PC = 784                   # S columns; e_local = c*64 + p, p in [0,64)
NT = EP // 64              # 784 primary scatter tiles (64-edge windows)
NG = 4                     # generic (overflow) scatter tiles
NSLOT = NT + NG            # 788 angle slot columns
SUP = 2048                 # edge super-block (columns per DMA)
BLK = 512                  # matmul moving width
PH1_CH = 16                # angle slot columns per phase-1 chunk

_PROG = None
LAST_RESULT = None


def _build_program():
    # CoreSim has no Silu; tests can force Sigmoid to validate dataflow.
    silu_f = (ACTF.Sigmoid if os.environ.get("KERNEL_SIM_ACT") == "sigmoid"
              else ACTF.Silu)
    nc = bacc.Bacc("TRN2", target_bir_lowering=False, debug=False,
                   num_devices=NCORES)

    a_d = nc.dram_tensor("a_arr", [128, NSLOT * ADIM], F16, kind="ExternalInput")
    r_d = nc.dram_tensor("r16", [128, NSLOT], F16, kind="ExternalInput")
    cg_d = nc.dram_tensor("cg16", [128, NG], F32, kind="ExternalInput")
    mT_d = nc.dram_tensor("mT0", [128, EP], F16, kind="ExternalInput")
    tail_d = nc.dram_tensor("tailT", [12, EP], F16, kind="ExternalInput")
    wm00_d = nc.dram_tensor("Wm00", [128, 128], F16, kind="ExternalInput")
    wmk1_d = nc.dram_tensor("WmK1M0", [6, 128], F16, kind="ExternalInput")
    wsa_d = nc.dram_tensor("WstragA", [128, 38], F16, kind="ExternalInput")
    wsb_d = nc.dram_tensor("WstragB", [12, 38], F16, kind="ExternalInput")
    we0_d = nc.dram_tensor("WeT0", [6, 128], F16, kind="ExternalInput")
    fw0_d = nc.dram_tensor("fw0", [128, 6], F16, kind="ExternalInput")
    fw1_d = nc.dram_tensor("fw1", [6, 6], F16, kind="ExternalInput")
    b0_d = nc.dram_tensor("b0", [128, 1], F32, kind="ExternalInput")
    b1_d = nc.dram_tensor("b1", [6, 1], F32, kind="ExternalInput")
    wsum_d = nc.dram_tensor("wsum_rep", [128, PH1_CH * ADIM], F16,
                            kind="ExternalInput")
    io64_d = nc.dram_tensor("iota8x64", [128, 512], F16, kind="ExternalInput")
    io64B_d = nc.dram_tensor("iota16x64", [128, 1024], F16, kind="ExternalInput")
    io128_d = nc.dram_tensor("iota128", [128, 128], F16, kind="ExternalInput")
    io392_d = nc.dram_tensor("iota784", [128, PC], F16, kind="ExternalInput")
    ones_d = nc.dram_tensor("ones128", [128, 1], F16, kind="ExternalInput")
    ident_d = nc.dram_tensor("ident128", [128, 128], F32, kind="ExternalInput")
    out_d = nc.dram_tensor("out", [6, EP], F16, kind="ExternalOutput")

    with tile.TileContext(nc) as tc:
        with tc.tile_pool(name="const", bufs=1) as cpool, \
             tc.tile_pool(name="dram", bufs=1, space="DRAM") as dpool, \
             tc.tile_pool(name="persist", bufs=1) as ppool:

            def cload(dram, shape, dtype=F16, tag=None):
                t = cpool.tile(shape, dtype, tag=tag or dram.name)
                nc.sync.dma_start(out=t[:], in_=dram[:])
                return t

            r_t = cload(r_d, [128, NSLOT])
            cg_t = cload(cg_d, [128, NG], F32)
            wm00_t = cload(wm00_d, [128, 128])
            wmk1_t = cload(wmk1_d, [6, 128])
            wsa_t = cload(wsa_d, [128, 38])
            wsb_t = cload(wsb_d, [12, 38])
            we0_t = cload(we0_d, [6, 128])
            fw0_t = cload(fw0_d, [128, 6])
            fw1_t = cload(fw1_d, [6, 6])
            b0_t = cload(b0_d, [128, 1], F32)
            b1_t = cload(b1_d, [6, 1], F32)
            wsum_t = cload(wsum_d, [128, PH1_CH * ADIM])
            io64_t = cload(io64_d, [128, 512])
            io64B_t = cload(io64B_d, [128, 1024])
            io128_t = cload(io128_d, [128, 128])
            io392_t = cload(io392_d, [128, PC])
            ones_t = cload(ones_d, [128, 1])
            ident_t = cload(ident_d, [128, 128], F32)

            s16 = ppool.tile([128, NSLOT], F16, tag="s16")
            S_sb = ppool.tile([64, PC], F32, tag="S_sb")
            S_sbT = ppool.tile([98, 512], F16, tag="S_sbT")
            S_dramT = dpool.tile([PC, 64], F16, tag="S_dramT")

            # ---- Phase 1: s[slot] = a_sbf[slot] . w_sum  -------------------
            with tc.tile_pool(name="ph1", bufs=3) as p1:
                for off in range(0, NSLOT, PH1_CH):
                    w = min(PH1_CH, NSLOT - off)
                    at = p1.tile([128, PH1_CH * ADIM], F16, tag="at")
                    nc.sync.dma_start(out=at[:, :w * ADIM],
                                      in_=a_d[:, off * ADIM:(off + w) * ADIM])
                    pr = p1.tile([128, PH1_CH * ADIM], F16, tag="pr")
                    nc.gpsimd.tensor_tensor(out=pr[:, :w * ADIM],
                                            in0=at[:, :w * ADIM],
                                            in1=wsum_t[:, :w * ADIM],
                                            op=OP.mult)
                    with nc.allow_low_precision("s fits fp16"):
                        nc.vector.tensor_reduce(
                            out=s16[:, off:off + w],
                            in_=pr[:, :w * ADIM].rearrange("p (t d) -> p t d",
                                                           d=ADIM),
                            axis=AX.X, op=OP.add)

            # ---- Phase 2: segment-sum (runs first; PE cost is small) -----
            proj_all = ppool.tile([6, EP], F16, tag="proj_all")
            supers = [(st0, min(SUP, EP - st0)) for st0 in range(0, EP, SUP)]
            B = 16
            with tc.tile_pool(name="ph2psum", bufs=1, space="PSUM") as sp, \
                 tc.tile_pool(name="ph2", bufs=6) as p2:
                S_ps = sp.tile([64, PC], F32, tag="S_ps")
                # One accumulation group per PSUM bank: a zeroing matmul
                # (start=True) covers the bank, scatter matmuls accumulate
                # (start=False), the last generic matmul carries stop=True.
                z1 = p2.tile([1, 64], F16, tag="z1")
                nc.gpsimd.memset(z1[:], 0)
                nc.tensor.matmul(S_ps[0:64, 0:512], z1[:], io64_t[0:1, :],
                                 start=True, stop=False)
                nc.tensor.matmul(S_ps[0:64, 512:PC], z1[:],
                                 io64_t[0:1, 0:PC - 512],
                                 start=True, stop=False)
                for grp in range(NT // B):
                    # Primary tile t covers local edges [64t, 64t+64); with
                    # el = c*64 + p this is psum column t, partitions 0:64.
                    # The eq tile is an unscaled one-hot; the multiply by s
                    # happens inside the matmul (rhs = s column).
                    eqB = p2.tile([128, B * 64], F16, tag="eqB")
                    nc.vector.tensor_tensor(
                        out=eqB[:].rearrange("p (t j) -> p t j", j=64),
                        in0=io64B_t[:].rearrange("p (t j) -> p t j", j=64),
                        in1=r_t[:, grp * B:(grp + 1) * B]
                            .to_broadcast([128, B, 64]),
                        op=OP.is_equal)
                    for i in range(B):
                        t = grp * B + i
                        nc.tensor.matmul(S_ps[0:64, t:t + 1],
                                         eqB[:, i * 64:(i + 1) * 64],
                                         s16[:, t:t + 1],
                                         start=False, stop=False)
                for g in range(NG):
                    pg = p2.tile([128, 64], F16, tag="pg")
                    nc.vector.scalar_tensor_tensor(
                        out=pg[:], in0=io64_t[:, 0:64],
                        scalar=r_t[:, NT + g:NT + g + 1],
                        in1=s16[:, NT + g:NT + g + 1].to_broadcast([128, 64]),
                        op0=OP.is_equal, op1=OP.mult)
                    cg = p2.tile([128, PC], F16, tag="cgt")
                    nc.vector.tensor_scalar(
                        out=cg[:], in0=io392_t[:],
                        scalar1=cg_t[:, g:g + 1], scalar2=None,
                        op0=OP.is_equal)
                    nc.tensor.matmul(S_ps[0:64, 0:512], pg[:], cg[:, 0:512],
                                     start=False, stop=(g == NG - 1))
                    nc.tensor.matmul(S_ps[0:64, 512:PC], pg[:],
                                     cg[:, 512:PC],
                                     start=False, stop=(g == NG - 1))
                nc.scalar.activation(out=S_sb[:], in_=S_ps[:], func=ACTF.Copy)
                # Transpose S [p, c] -> [c, p] so DRAM flat order == el order.
                for q in range(8):
                    T_ps = sp.tile([98, 64], F32, tag="S_ps")
                    nc.tensor.transpose(out=T_ps[:],
                                        in_=S_sb[0:64, 98 * q:98 * (q + 1)],
                                        identity=ident_t[0:64, 0:64])
                    nc.scalar.activation(out=S_sbT[:, 64 * q:64 * (q + 1)],
                                         in_=T_ps[:], func=ACTF.Copy)
            nc.sync.dma_start(
                out=S_dramT[:].rearrange("(q r) p -> r q p", q=8),
                in_=S_sbT[:].rearrange("r (q p) -> r q p", q=8))
            S_flat = S_dramT[:].rearrange("(o c) p -> o (c p)", o=1)

            # ---- Phase 3: h(e), then the S-scale sweep --------------------
            with tc.tile_pool(name="pm", bufs=2, space="PSUM") as pmp, \
                 tc.tile_pool(name="st", bufs=2, space="PSUM") as stp, \
                 tc.tile_pool(name="te", bufs=2, space="PSUM") as tep, \
                 tc.tile_pool(name="pj", bufs=2, space="PSUM") as pjp, \
                 tc.tile_pool(name="ph3", bufs=3) as p3:
                for st0, wd in supers:
                    mt = p3.tile([128, SUP], F16, tag="mt")
                    nc.sync.dma_start(out=mt[:, :wd], in_=mT_d[:, st0:st0 + wd])
                    tl = p3.tile([12, SUP], F16, tag="tl")
                    nc.sync.dma_start(out=tl[:, :wd],
                                      in_=tail_d[:, st0:st0 + wd])
                    et = p3.tile([6, SUP], F16, tag="et")
                    nc.sync.dma_start(out=et[:, :wd],
                                      in_=tail_d[6:12, st0:st0 + wd])
                    for lo in range(0, wd, BLK):
                        sl = slice(lo, lo + BLK)
                        pm0 = pmp.tile([128, BLK], F32, tag="pm0")
                        nc.tensor.matmul(pm0[:], wm00_t[:], mt[:, sl],
                                         start=True, stop=False)
                        nc.tensor.matmul(pm0[:], wmk1_t[:], tl[0:6, sl],
                                         start=False, stop=True)
                        stg = stp.tile([38, BLK], F32, tag="stg")
                        nc.tensor.matmul(stg[:], wsa_t[:], mt[:, sl],
                                         start=True, stop=False)
                        nc.tensor.matmul(stg[:], wsb_t[:], tl[0:12, sl],
                                         start=False, stop=True)
                        te0 = tep.tile([128, BLK], F32, tag="te0")
                        nc.tensor.matmul(te0[:], we0_t[:], et[:, sl],
                                         start=True, stop=True)
                        m0 = p3.tile([128, BLK], F16, tag="m0")
                        nc.scalar.activation(out=m0[:], in_=pm0[:],
                                             func=silu_f, bias=b0_t[:, 0:1])
                        m1 = p3.tile([6, BLK], F16, tag="m1")
                        nc.scalar.activation(out=m1[:], in_=stg[0:6, :],
                                             func=silu_f, bias=b1_t[:, 0:1])
                        t0 = p3.tile([128, BLK], F16, tag="t0")
                        nc.scalar.activation(out=t0[:], in_=te0[:],
                                             func=ACTF.Copy)
                        me0 = p3.tile([128, BLK], F16, tag="me0")
                        nc.vector.tensor_tensor(out=me0[:], in0=m0[:],
                                                in1=t0[:], op=OP.mult)
                        me1 = p3.tile([6, BLK], F16, tag="me1")
                        nc.vector.tensor_tensor(out=me1[:], in0=m1[:],
                                                in1=stg[32:38, :], op=OP.mult)
                        pj = pjp.tile([6, BLK], F32, tag="pj")
                        nc.tensor.matmul(pj[:], fw0_t[:], me0[:],
                                         start=True, stop=False)
                        nc.tensor.matmul(pj[:], fw1_t[:], me1[:],
                                         start=False, stop=True)
                        nc.scalar.activation(
                            out=proj_all[:, st0 + lo:st0 + lo + BLK],
                            in_=pj[:], func=ACTF.Copy)

                # Final S-scale sweep over the whole shard.
                for st0, wd in supers:
                    s6 = p3.tile([6, SUP], F16, tag="s6")
                    nc.sync.dma_start(out=s6[:, :wd],
                                      in_=S_flat[0:1, st0:st0 + wd]
                                          .to_broadcast([6, wd]))
                    ob = p3.tile([6, SUP], F16, tag="ob")
                    nc.vector.tensor_tensor(out=ob[:, :wd],
                                            in0=proj_all[:, st0:st0 + wd],
                                            in1=s6[:, :wd], op=OP.mult)
                    nc.sync.dma_start(out=out_d[:, st0:st0 + wd],
                                      in_=ob[:, :wd])

    nc.compile()
    return nc


def _get_program():
    global _PROG
    if _PROG is None:
        _PROG = _build_program()
    return _PROG


import ml_dtypes


def _f16(x):
    return np.ascontiguousarray(x, dtype=np.float16)


def _bf16(x):
    return np.ascontiguousarray(np.asarray(x, np.float32).astype(ml_dtypes.bfloat16))


def _prep_inputs(m_ji, e_rbf, a_sbf, kj_idx, W_m, b_m, W_e, W_a, final_w):
    m_ji = np.asarray(m_ji, dtype=np.float32)
    e_rbf = np.asarray(e_rbf, dtype=np.float32)
    a_sbf = np.asarray(a_sbf, dtype=np.float32)
    kj = np.asarray(kj_idx).astype(np.int64).ravel()
    W_m = np.asarray(W_m, dtype=np.float32)
    b_m = np.asarray(b_m, dtype=np.float32).ravel()
    W_e = np.asarray(W_e, dtype=np.float32)
    W_a = np.asarray(W_a, dtype=np.float32)
    fw = np.asarray(final_w, dtype=np.float32)

    # shared weight-derived inputs
    WmT = W_m.T  # [c_in, c_out]
    WeT = W_e.T  # [6?? no: W_e is [CAT, NRBF]] -> W_e.T is [NRBF, CAT]
    fwT = fw.T   # [CAT, NRBF]
    wsa = np.zeros((128, 38), np.float32)
    wsa[:, 0:6] = WmT[:128, 128:134]
    wsb = np.zeros((12, 38), np.float32)
    wsb[0:6, 0:6] = WmT[128:134, 128:134]
    # te[c_out, e] = sum_j W_e[c_out, j] * e_rbf[e, j]; lhsT[k=j, m=c_out]
    # = W_e.T[j, c_out] = WeT[j, c_out] with WeT = W_e.T  ([NRBF, CAT])
    # te tail lands on psum partitions 32:38 (32-aligned engine reads).
    wsb[6:12, 32:38] = WeT[:, 128:134]
    w_sum = W_a.sum(axis=0)  # [42]

    shared = {
        "Wm00": _f16(WmT[:128, :128]),
        "WmK1M0": _f16(WmT[128:134, :128]),
        "WstragA": _f16(wsa),
        "WstragB": _f16(wsb),
        "WeT0": _f16(WeT[:, :128]),
        "fw0": _f16(fwT[:128, :]),
        "fw1": _f16(fwT[128:134, :]),
        "b0": np.ascontiguousarray(b_m[:128, None], np.float32),
        "b1": np.ascontiguousarray(b_m[128:134, None], np.float32),
        "wsum_rep": _f16(np.tile(w_sum, (128, PH1_CH))),
        "iota8x64": _f16(np.tile(np.arange(64), (128, 8))),
        "iota16x64": _f16(np.tile(np.arange(64), (128, 16))),
        "iota128": _f16(np.tile(np.arange(128), (128, 1))),
        "iota784": _f16(np.tile(np.arange(PC), (128, 1))),
        "ones128": _f16(np.ones((128, 1))),
        "ident128": np.eye(128, dtype=np.float32),
    }

    order = np.argsort(kj, kind="stable")
    kj_s = kj[order]
    a_s = a_sbf[order]
    bounds = np.searchsorted(kj_s, np.arange(NCORES + 1) * ESH)

    in_maps = []
    for i in range(NCORES):
        lo, hi = int(bounds[i]), int(bounds[i + 1])
        el = kj_s[lo:hi] - i * ESH          # sorted ascending in [0, ESH)
        av = a_s[lo:hi]
        n = el.shape[0]
        t_of = el // 64
        starts = np.searchsorted(t_of, np.arange(NT))
        rank = np.arange(n) - starts[t_of]
        prim = rank < 128
        r16 = np.zeros((128, NSLOT), np.float16)
        a_arr = np.zeros((128, NSLOT, ADIM), np.float16)
        cg16 = np.zeros((128, NG), np.float32)
        r16[rank[prim], t_of[prim]] = (el % 64)[prim]
        a_arr[rank[prim], t_of[prim], :] = av[prim]
        n_sp = int((~prim).sum())
        if n_sp > NG * 128:
            raise RuntimeError(f"core {i}: {n_sp} spill angles > {NG*128}")
        if n_sp:
            sp_el = el[~prim]
            sp_idx = np.arange(n_sp)
            rr, cc = sp_idx % 128, sp_idx // 128
            r16[rr, NT + cc] = sp_el % 64
            cg16[rr, cc] = sp_el // 64
            a_arr[rr, NT + cc, :] = av[~prim]

        msl = m_ji[i * ESH:(i + 1) * ESH]
        mT = np.zeros((CAT, EP), np.float16)
        mT[:, :ESH] = msl.T
        esl = e_rbf[i * ESH:(i + 1) * ESH]
        tailT = np.zeros((12, EP), np.float16)
        tailT[0:6, :ESH] = msl.T[128:134]
        tailT[6:12, :ESH] = esl.T

        im = dict(shared)
        im["a_arr"] = np.ascontiguousarray(a_arr.reshape(128, NSLOT * ADIM))
        im["r16"] = np.ascontiguousarray(r16)
        im["cg16"] = np.ascontiguousarray(cg16)
        im["mT0"] = np.ascontiguousarray(mT[:128])
        im["tailT"] = np.ascontiguousarray(tailT)
        in_maps.append(im)
    return in_maps


def kernel(m_ji, nbr_list, angle_list, e_rbf, a_sbf, kj_idx,
           W_m, b_m, W_e, W_a, final_w):
    global LAST_RESULT
    in_maps = _prep_inputs(m_ji, e_rbf, a_sbf, kj_idx, W_m, b_m, W_e, W_a,
                           final_w)
    nc = _get_program()
    res = run_bass_kernel_spmd(nc, in_maps, core_ids=list(range(NCORES)))
    LAST_RESULT = res
    out = np.empty((E, NRBF), np.float32)
    for i in range(NCORES):
        out[i * ESH:(i + 1) * ESH] = res.results[i]["out"][:, :ESH].T.astype(np.float32)
    return out


# revision 25
# speedup vs baseline: 1.5111x; 1.2837x over previous
"""Trainium2 Bass kernel for nn_DirectedMessage (gnn_message_passing).

Math: the reference's per-angle tensor m_and_e depends only on kj_idx[a], so
    final[e] = h(e) * S(e)
      h(e) = (silu(m_ji[e] @ W_m.T + b) * (e_rbf[e] @ W_e.T)) @ final_w.T   [E, 6]
      s[a] = a_sbf[a] . sum_r W_a[r]                                        [A]
      S(e) = segment_sum(s, kj_idx)[e]                                      [E]

Distribution (owner-computes): edges are sharded contiguously across the 8
cores; each angle is routed (on host, as part of sharding) to the core that
owns its kj edge, so no collective is needed.  Within a core, angles are
binned into fixed 64-edge windows (tile t covers local edges [64t, 64t+64));
the device computes s on-chip and performs the segment-sum with one small
PSUM-accumulating matmul per tile (lhsT = constant ones column, rhs = a
one-hot-times-s matrix built on the vector engine).  Overflow angles (>128
in one window) go through 4 generic full-width scatter tiles.
"""

import sys
import types

sys.path.insert(0, "/opt/trn_rl_repo")

# Optional NTFF trace hook (lets BASS_TRACE=1 capture hardware profiles).
try:  # pragma: no cover
    import trn_agent_boot.trn_boot as _tb

    if "antenv.axon_hooks" not in sys.modules:
        _hook = _tb._ntff_profile_via_ctypes("/opt/axon/libaxon_pjrt.so")
        _m = types.ModuleType("antenv.axon_hooks")
        _m.get_axon_ntff_profile_hook = lambda: _hook
        sys.modules["antenv.axon_hooks"] = _m
except Exception:
    pass

import os

import numpy as np

import concourse.bacc as bacc
import concourse.mybir as mybir
import concourse.tile as tile
from concourse.bass_utils import run_bass_kernel_spmd

F16 = mybir.dt.float16
BF16 = mybir.dt.bfloat16
F32 = mybir.dt.float32
OP = mybir.AluOpType
ACTF = mybir.ActivationFunctionType
AX = mybir.AxisListType

E = 400000
A = 600000
CAT = 134
NRBF = 6
ADIM = 42
NCORES = 8
ESH = E // NCORES          # 50000 edges per core
PW = 392                   # legacy name; see NT/PC below
EP = 50176                 # padded edges per core
PC = 784                   # S columns; e_local = c*64 + p, p in [0,64)
NT = EP // 64              # 784 primary scatter tiles (64-edge windows)
NG = 4                     # generic (overflow) scatter tiles
NSLOT = NT + NG            # 788 angle slot columns
SUP = 2048                 # edge super-block (columns per DMA)
BLK = 512                  # matmul moving width
PH1_CH = 16                # angle slot columns per phase-1 chunk

_PROG = None
LAST_RESULT = None


def _build_program():
    # CoreSim has no Silu; tests can force Sigmoid to validate dataflow.
    silu_f = (ACTF.Sigmoid if os.environ.get("KERNEL_SIM_ACT") == "sigmoid"
              else ACTF.Silu)
    nc = bacc.Bacc("TRN2", target_bir_lowering=False, debug=False,
                   num_devices=NCORES)

    a_d = nc.dram_tensor("a_arr", [128, NSLOT * ADIM], F16, kind="ExternalInput")
    r_d = nc.dram_tensor("r16", [128, NSLOT], F16, kind="ExternalInput")
    cg_d = nc.dram_tensor("cg16", [128, NG], F32, kind="ExternalInput")
    mT_d = nc.dram_tensor("mT0", [128, EP], F16, kind="ExternalInput")
    tail_d = nc.dram_tensor("tailT", [12, EP], F16, kind="ExternalInput")
    wm00_d = nc.dram_tensor("Wm00", [128, 128], F16, kind="ExternalInput")
    wmk1_d = nc.dram_tensor("WmK1M0", [128, 128], F16, kind="ExternalInput")
    wsa_d = nc.dram_tensor("WstragA", [128, 38], F16, kind="ExternalInput")
    wsb_d = nc.dram_tensor("WstragB", [128, 38], F16, kind="ExternalInput")
    we0_d = nc.dram_tensor("WeT0", [128, 128], F16, kind="ExternalInput")
    fw0_d = nc.dram_tensor("fw0", [128, 6], F16, kind="ExternalInput")
    fw1_d = nc.dram_tensor("fw1", [128, 6], F16, kind="ExternalInput")
    b0_d = nc.dram_tensor("b0", [128, 1], F32, kind="ExternalInput")
    b1_d = nc.dram_tensor("b1", [6, 1], F32, kind="ExternalInput")
    wsum_d = nc.dram_tensor("wsum_rep", [128, PH1_CH * ADIM], F16,
                            kind="ExternalInput")
    io64_d = nc.dram_tensor("iota8x64", [128, 512], F16, kind="ExternalInput")
    io64B_d = nc.dram_tensor("iota16x64", [128, 1024], F16, kind="ExternalInput")
    io128_d = nc.dram_tensor("iota128", [128, 128], F16, kind="ExternalInput")
    io392_d = nc.dram_tensor("iota784", [128, PC], F16, kind="ExternalInput")
    ones_d = nc.dram_tensor("ones128", [128, 1], F16, kind="ExternalInput")
    ident_d = nc.dram_tensor("ident128", [128, 128], F32, kind="ExternalInput")
    out_d = nc.dram_tensor("out", [6, EP], F16, kind="ExternalOutput")

    with tile.TileContext(nc) as tc:
        with tc.tile_pool(name="const", bufs=1) as cpool, \
             tc.tile_pool(name="dram", bufs=1, space="DRAM") as dpool, \
             tc.tile_pool(name="persist", bufs=1) as ppool:

            def cload(dram, shape, dtype=F16, tag=None):
                t = cpool.tile(shape, dtype, tag=tag or dram.name)
                nc.sync.dma_start(out=t[:], in_=dram[:])
                return t

            r_t = cload(r_d, [128, NSLOT])
            cg_t = cload(cg_d, [128, NG], F32)
            wm00_t = cload(wm00_d, [128, 128])
            wmk1_t = cload(wmk1_d, [128, 128])
            wsa_t = cload(wsa_d, [128, 38])
            wsb_t = cload(wsb_d, [128, 38])
            we0_t = cload(we0_d, [128, 128])
            fw0_t = cload(fw0_d, [128, 6])
            fw1_t = cload(fw1_d, [128, 6])
            b0_t = cload(b0_d, [128, 1], F32)
            b1_t = cload(b1_d, [6, 1], F32)
            wsum_t = cload(wsum_d, [128, PH1_CH * ADIM])
            io64_t = cload(io64_d, [128, 512])
            io64B_t = cload(io64B_d, [128, 1024])
            io128_t = cload(io128_d, [128, 128])
            io392_t = cload(io392_d, [128, PC])
            ones_t = cload(ones_d, [128, 1])
            ident_t = cload(ident_d, [128, 128], F32)

            s16 = ppool.tile([128, NSLOT], F16, tag="s16")
            tl128 = []
            et128 = []
            for pi in range(3):
                tlp = ppool.tile([128, SUP], F16, tag=f"tl128_{pi}")
                nc.gpsimd.memset(tlp[:], 0)
                tl128.append(tlp)
                etp = ppool.tile([128, SUP], F16, tag=f"et128_{pi}")
                nc.gpsimd.memset(etp[:], 0)
                et128.append(etp)
            me128 = []
            for pi in range(2):
                mep = ppool.tile([128, BLK], F16, tag=f"me128_{pi}")
                nc.gpsimd.memset(mep[:], 0)
                me128.append(mep)
            S_sb = ppool.tile([64, PC], F32, tag="S_sb")
            S_sbT = ppool.tile([98, 512], F16, tag="S_sbT")
            S_dramT = dpool.tile([PC, 64], F16, tag="S_dramT")

            # ---- Phase 1: s[slot] = a_sbf[slot] . w_sum  -------------------
            with tc.tile_pool(name="ph1", bufs=3) as p1:
                for off in range(0, NSLOT, PH1_CH):
                    w = min(PH1_CH, NSLOT - off)
                    at = p1.tile([128, PH1_CH * ADIM], F16, tag="at")
                    nc.sync.dma_start(out=at[:, :w * ADIM],
                                      in_=a_d[:, off * ADIM:(off + w) * ADIM])
                    pr = p1.tile([128, PH1_CH * ADIM], F16, tag="pr")
                    nc.gpsimd.tensor_tensor(out=pr[:, :w * ADIM],
                                            in0=at[:, :w * ADIM],
                                            in1=wsum_t[:, :w * ADIM],
                                            op=OP.mult)
                    with nc.allow_low_precision("s fits fp16"):
                        nc.vector.tensor_reduce(
                            out=s16[:, off:off + w],
                            in_=pr[:, :w * ADIM].rearrange("p (t d) -> p t d",
                                                           d=ADIM),
                            axis=AX.X, op=OP.add)

            # ---- Phase 2: segment-sum (runs first; PE cost is small) -----
            proj_all = ppool.tile([6, EP], F16, tag="proj_all")
            supers = [(st0, min(SUP, EP - st0)) for st0 in range(0, EP, SUP)]
            B = 16
            with tc.tile_pool(name="ph2psum", bufs=1, space="PSUM") as sp, \
                 tc.tile_pool(name="ph2", bufs=6) as p2:
                S_ps = sp.tile([64, PC], F32, tag="S_ps")
                # One accumulation group per PSUM bank: a zeroing matmul
                # (start=True) covers the bank, scatter matmuls accumulate
                # (start=False), the last generic matmul carries stop=True.
                z1 = p2.tile([1, 64], F16, tag="z1")
                nc.gpsimd.memset(z1[:], 0)
                nc.tensor.matmul(S_ps[0:64, 0:512], z1[:], io64_t[0:1, :],
                                 start=True, stop=False)
                nc.tensor.matmul(S_ps[0:64, 512:PC], z1[:],
                                 io64_t[0:1, 0:PC - 512],
                                 start=True, stop=False)
                for grp in range(NT // B):
                    # Primary tile t covers local edges [64t, 64t+64); with
                    # el = c*64 + p this is psum column t, partitions 0:64.
                    # The eq tile is an unscaled one-hot; the multiply by s
                    # happens inside the matmul (rhs = s column).
                    eqB = p2.tile([128, B * 64], F16, tag="eqB")
                    nc.vector.tensor_tensor(
                        out=eqB[:].rearrange("p (t j) -> p t j", j=64),
                        in0=io64B_t[:].rearrange("p (t j) -> p t j", j=64),
                        in1=r_t[:, grp * B:(grp + 1) * B]
                            .to_broadcast([128, B, 64]),
                        op=OP.is_equal)
                    for i in range(B):
                        t = grp * B + i
                        nc.tensor.matmul(S_ps[0:64, t:t + 1],
                                         eqB[:, i * 64:(i + 1) * 64],
                                         s16[:, t:t + 1],
                                         start=False, stop=False)
                for g in range(NG):
                    pg = p2.tile([128, 64], F16, tag="pg")
                    nc.vector.scalar_tensor_tensor(
                        out=pg[:], in0=io64_t[:, 0:64],
                        scalar=r_t[:, NT + g:NT + g + 1],
                        in1=s16[:, NT + g:NT + g + 1].to_broadcast([128, 64]),
                        op0=OP.is_equal, op1=OP.mult)
                    cg = p2.tile([128, PC], F16, tag="cgt")
                    nc.vector.tensor_scalar(
                        out=cg[:], in0=io392_t[:],
                        scalar1=cg_t[:, g:g + 1], scalar2=None,
                        op0=OP.is_equal)
                    nc.tensor.matmul(S_ps[0:64, 0:512], pg[:], cg[:, 0:512],
                                     start=False, stop=(g == NG - 1))
                    nc.tensor.matmul(S_ps[0:64, 512:PC], pg[:],
                                     cg[:, 512:PC],
                                     start=False, stop=(g == NG - 1))
                nc.scalar.activation(out=S_sb[:], in_=S_ps[:], func=ACTF.Copy)
                # Transpose S [p, c] -> [c, p] so DRAM flat order == el order.
                for q in range(8):
                    T_ps = sp.tile([98, 64], F32, tag="S_ps")
                    nc.tensor.transpose(out=T_ps[:],
                                        in_=S_sb[0:64, 98 * q:98 * (q + 1)],
                                        identity=ident_t[0:64, 0:64])
                    nc.scalar.activation(out=S_sbT[:, 64 * q:64 * (q + 1)],
                                         in_=T_ps[:], func=ACTF.Copy)
            nc.sync.dma_start(
                out=S_dramT[:].rearrange("(q r) p -> r q p", q=8),
                in_=S_sbT[:].rearrange("r (q p) -> r q p", q=8))
            S_flat = S_dramT[:].rearrange("(o c) p -> o (c p)", o=1)

            # ---- Phase 3: h(e), then the S-scale sweep --------------------
            with tc.tile_pool(name="pm", bufs=2, space="PSUM") as pmp, \
                 tc.tile_pool(name="st", bufs=2, space="PSUM") as stp, \
                 tc.tile_pool(name="te", bufs=2, space="PSUM") as tep, \
                 tc.tile_pool(name="pj", bufs=2, space="PSUM") as pjp, \
                 tc.tile_pool(name="ph3", bufs=3) as p3:
                for si, (st0, wd) in enumerate(supers):
                    mt = p3.tile([128, SUP], F16, tag="mt")
                    nc.sync.dma_start(out=mt[:, :wd], in_=mT_d[:, st0:st0 + wd])
                    tl = tl128[si % 3]
                    nc.sync.dma_start(out=tl[0:12, :wd],
                                      in_=tail_d[:, st0:st0 + wd])
                    et = et128[si % 3]
                    nc.sync.dma_start(out=et[0:6, :wd],
                                      in_=tail_d[6:12, st0:st0 + wd])
                    for bi, lo in enumerate(range(0, wd, BLK)):
                        sl = slice(lo, lo + BLK)
                        pm0 = pmp.tile([128, BLK], F32, tag="pm0")
                        nc.tensor.matmul(pm0[:], wm00_t[:], mt[:, sl],
                                         start=True, stop=False)
                        nc.tensor.matmul(pm0[:], wmk1_t[:], tl[:, sl],
                                         start=False, stop=True)
                        stg = stp.tile([38, BLK], F32, tag="stg")
                        nc.tensor.matmul(stg[:], wsa_t[:], mt[:, sl],
                                         start=True, stop=False)
                        nc.tensor.matmul(stg[:], wsb_t[:], tl[:, sl],
                                         start=False, stop=True)
                        te0 = tep.tile([128, BLK], F32, tag="te0")
                        nc.tensor.matmul(te0[:], we0_t[:], et[:, sl],
                                         start=True, stop=True)
                        m0 = p3.tile([128, BLK], F16, tag="m0")
                        nc.scalar.activation(out=m0[:], in_=pm0[:],
                                             func=silu_f, bias=b0_t[:, 0:1])
                        m1 = p3.tile([6, BLK], F16, tag="m1")
                        nc.scalar.activation(out=m1[:], in_=stg[0:6, :],
                                             func=silu_f, bias=b1_t[:, 0:1])
                        t0 = p3.tile([128, BLK], F16, tag="t0")
                        nc.scalar.activation(out=t0[:], in_=te0[:],
                                             func=ACTF.Copy)
                        me0 = p3.tile([128, BLK], F16, tag="me0")
                        nc.vector.tensor_tensor(out=me0[:], in0=m0[:],
                                                in1=t0[:], op=OP.mult)
                        me1 = me128[(si * 4 + bi) % 2]
                        nc.vector.tensor_tensor(out=me1[0:6, :], in0=m1[:],
                                                in1=stg[32:38, :], op=OP.mult)
                        pj = pjp.tile([6, BLK], F32, tag="pj")
                        nc.tensor.matmul(pj[:], fw0_t[:], me0[:],
                                         start=True, stop=False)
                        nc.tensor.matmul(pj[:], fw1_t[:], me1[:, :],
                                         start=False, stop=True)
                        nc.scalar.activation(
                            out=proj_all[:, st0 + lo:st0 + lo + BLK],
                            in_=pj[:], func=ACTF.Copy)

                # Final S-scale sweep over the whole shard.
                for st0, wd in supers:
                    s6 = p3.tile([6, SUP], F16, tag="s6")
                    nc.sync.dma_start(out=s6[:, :wd],
                                      in_=S_flat[0:1, st0:st0 + wd]
                                          .to_broadcast([6, wd]))
                    ob = p3.tile([6, SUP], F16, tag="ob")
                    nc.vector.tensor_tensor(out=ob[:, :wd],
                                            in0=proj_all[:, st0:st0 + wd],
                                            in1=s6[:, :wd], op=OP.mult)
                    nc.sync.dma_start(out=out_d[:, st0:st0 + wd],
                                      in_=ob[:, :wd])

    nc.compile()
    return nc


def _get_program():
    global _PROG
    if _PROG is None:
        _PROG = _build_program()
    return _PROG


import ml_dtypes


def _f16(x):
    return np.ascontiguousarray(x, dtype=np.float16)


def _bf16(x):
    return np.ascontiguousarray(np.asarray(x, np.float32).astype(ml_dtypes.bfloat16))


def _prep_inputs(m_ji, e_rbf, a_sbf, kj_idx, W_m, b_m, W_e, W_a, final_w):
    m_ji = np.asarray(m_ji, dtype=np.float32)
    e_rbf = np.asarray(e_rbf, dtype=np.float32)
    a_sbf = np.asarray(a_sbf, dtype=np.float32)
    kj = np.asarray(kj_idx).astype(np.int64).ravel()
    W_m = np.asarray(W_m, dtype=np.float32)
    b_m = np.asarray(b_m, dtype=np.float32).ravel()
    W_e = np.asarray(W_e, dtype=np.float32)
    W_a = np.asarray(W_a, dtype=np.float32)
    fw = np.asarray(final_w, dtype=np.float32)

    # shared weight-derived inputs
    WmT = W_m.T  # [c_in, c_out]
    WeT = W_e.T  # [6?? no: W_e is [CAT, NRBF]] -> W_e.T is [NRBF, CAT]
    fwT = fw.T   # [CAT, NRBF]
    wsa = np.zeros((128, 38), np.float32)
    wsa[:, 0:6] = WmT[:128, 128:134]
    wsb = np.zeros((12, 38), np.float32)
    wsb[0:6, 0:6] = WmT[128:134, 128:134]
    # te[c_out, e] = sum_j W_e[c_out, j] * e_rbf[e, j]; lhsT[k=j, m=c_out]
    # = W_e.T[j, c_out] = WeT[j, c_out] with WeT = W_e.T  ([NRBF, CAT])
    # te tail lands on psum partitions 32:38 (32-aligned engine reads).
    wsb[6:12, 32:38] = WeT[:, 128:134]
    w_sum = W_a.sum(axis=0)  # [42]

    wmk1p = np.zeros((128, 128), np.float32)
    wmk1p[0:6, :] = WmT[128:134, :128]
    wsbp = np.zeros((128, 38), np.float32)
    wsbp[:12, :] = wsb
    we0p = np.zeros((128, 128), np.float32)
    we0p[0:6, :] = WeT[:, :128]
    fw1p = np.zeros((128, 6), np.float32)
    fw1p[0:6, :] = fwT[128:134, :]
    shared = {
        "Wm00": _f16(WmT[:128, :128]),
        "WmK1M0": _f16(wmk1p),
        "WstragA": _f16(wsa),
        "WstragB": _f16(wsbp),
        "WeT0": _f16(we0p),
        "fw0": _f16(fwT[:128, :]),
        "fw1": _f16(fw1p),
        "b0": np.ascontiguousarray(b_m[:128, None], np.float32),
        "b1": np.ascontiguousarray(b_m[128:134, None], np.float32),
        "wsum_rep": _f16(np.tile(w_sum, (128, PH1_CH))),
        "iota8x64": _f16(np.tile(np.arange(64), (128, 8))),
        "iota16x64": _f16(np.tile(np.arange(64), (128, 16))),
        "iota128": _f16(np.tile(np.arange(128), (128, 1))),
        "iota784": _f16(np.tile(np.arange(PC), (128, 1))),
        "ones128": _f16(np.ones((128, 1))),
        "ident128": np.eye(128, dtype=np.float32),
    }

    order = np.argsort(kj, kind="stable")
    kj_s = kj[order]
    a_s = a_sbf[order]
    bounds = np.searchsorted(kj_s, np.arange(NCORES + 1) * ESH)

    in_maps = []
    for i in range(NCORES):
        lo, hi = int(bounds[i]), int(bounds[i + 1])
        el = kj_s[lo:hi] - i * ESH          # sorted ascending in [0, ESH)
        av = a_s[lo:hi]
        n = el.shape[0]
        t_of = el // 64
        starts = np.searchsorted(t_of, np.arange(NT))
        rank = np.arange(n) - starts[t_of]
        prim = rank < 128
        r16 = np.zeros((128, NSLOT), np.float16)
        a_arr = np.zeros((128, NSLOT, ADIM), np.float16)
        cg16 = np.zeros((128, NG), np.float32)
        r16[rank[prim], t_of[prim]] = (el % 64)[prim]
        a_arr[rank[prim], t_of[prim], :] = av[prim]
        n_sp = int((~prim).sum())
        if n_sp > NG * 128:
            raise RuntimeError(f"core {i}: {n_sp} spill angles > {NG*128}")
        if n_sp:
            sp_el = el[~prim]
            sp_idx = np.arange(n_sp)
            rr, cc = sp_idx % 128, sp_idx // 128
            r16[rr, NT + cc] = sp_el % 64
            cg16[rr, cc] = sp_el // 64
            a_arr[rr, NT + cc, :] = av[~prim]

        msl = m_ji[i * ESH:(i + 1) * ESH]
        mT = np.zeros((CAT, EP), np.float16)
        mT[:, :ESH] = msl.T
        esl = e_rbf[i * ESH:(i + 1) * ESH]
        tailT = np.zeros((12, EP), np.float16)
        tailT[0:6, :ESH] = msl.T[128:134]
        tailT[6:12, :ESH] = esl.T

        im = dict(shared)
        im["a_arr"] = np.ascontiguousarray(a_arr.reshape(128, NSLOT * ADIM))
        im["r16"] = np.ascontiguousarray(r16)
        im["cg16"] = np.ascontiguousarray(cg16)
        im["mT0"] = np.ascontiguousarray(mT[:128])
        im["tailT"] = np.ascontiguousarray(tailT)
        in_maps.append(im)
    return in_maps


def kernel(m_ji, nbr_list, angle_list, e_rbf, a_sbf, kj_idx,
           W_m, b_m, W_e, W_a, final_w):
    global LAST_RESULT
    in_maps = _prep_inputs(m_ji, e_rbf, a_sbf, kj_idx, W_m, b_m, W_e, W_a,
                           final_w)
    nc = _get_program()
    res = run_bass_kernel_spmd(nc, in_maps, core_ids=list(range(NCORES)))
    LAST_RESULT = res
    out = np.empty((E, NRBF), np.float32)
    for i in range(NCORES):
        out[i * ESH:(i + 1) * ESH] = res.results[i]["out"][:, :ESH].T.astype(np.float32)
    return out
